# revision 2
# baseline (speedup 1.0000x reference)
"""Two-layer GAT on 8 TRN2 cores — redesigned for gather/vector throughput.

Key changes vs baseline kernel.py:
- One merged row gather per edge per layer ([h|as] in-row); ad via one-hot
  matmul from an SBUF table of the core's own dst nodes (no ad gathers).
- One-hot matrices G01/G01T precomputed on host (index-only data), streamed
  densely via HWDGE — no per-chunk is_equal on DVE.
- Host passes xT/W0perm/W1perm layouts so phase A needs no PE transposes;
  phase-A PSUM results are written straight to DRAM with cast DMAs in
  4-tile batches.
- Leaky-relu/exp/1/z moved to the ACT engine (Lrelu/Exp/Ln); msg multiply is
  the only big DVE op per chunk.
- Layer-0 feature order is interleaved (f' = c*8 + h) so the per-edge alpha
  broadcast has unit inner stride.
- Gathers use prepare_only+trigger_dma on rotating SWDGE queues (cfg).
"""

import numpy as np
import ml_dtypes

import concourse.bass as bass
import concourse.tile as tile
from concourse import mybir
from concourse.vector_clock import ScopedClock
from concourse.masks import make_identity
from concourse.bass_utils import run_bass_kernel_spmd
from concourse.library_overlay import lower_extended_insts
from concourse import library_config

F32 = mybir.dt.float32
BF16 = mybir.dt.bfloat16
I16 = mybir.dt.int16
AF = mybir.ActivationFunctionType
ALU = mybir.AluOpType

NCORES = 8
N, E, F_IN, C, H = 20000, 320000, 128, 64, 8
NEG_SLOPE = 0.2
PN = N // NCORES
CHUNK = 128
CPB = 16
EB = CHUNK * CPB
ROW0 = 640            # bf16: [h 512 | as 8 | ad 8 | pad] -> 1280 B rows
ROW1 = 128            # bf16: [h1 64 | as1 | ad1 | pad] -> 256 B rows
NT0 = (N + 127) // 128          # global node tiles (157)
NTL = (PN + 127) // 128         # local dst tiles per core (20)
HROWS = NT0 * 128 + 8           # hcat row allocation (full-tile writes)

CFG = {"mode": "normal", "nq": 4, "pf": 2, "d_ap": False}


class FixedTileContext(tile.TileContext):
    """Walrus here rejects >1 sem wait on the tail Drain: hoist onto NOPs."""

    def _drain_and_barrier(self, tick_clock, wait_clock):
        nop = self.nc.sync.nop(nofuse=True, hint="pre_drain_waits")
        wait_clock.add_sem_waits(nop.ins, ScopedClock({None: tick_clock.global_clock}))
        si = nop.ins.sync_info
        waits = list(si.on_wait) if si and si.on_wait else []
        if len(waits) > 1:
            si.on_wait = [waits[0]]
            for w in waits[1:]:
                n2 = self.nc.sync.nop(nofuse=True, hint="pre_drain_waits")
                n2.ins.sync_info = mybir.SyncInfo(on_wait=[w], on_update=[])
        self.nc.sync.drain()
        self.nc.all_engine_barrier()
        popped = self.nc._tile_sem_poison_stack.pop()
        assert popped is self._sem_poison
        self.nc.clear_and_free_semaphores(list(self.sems.allocated().values()))
        self.nc.all_engine_barrier()


def _legalize_multi_waits(nc, limit=1):
    n_split = 0
    pre = {}
    made = set()
    blocks = [bb for f in nc.m.functions for bb in f.blocks]
    for bb in blocks:
        for inst in list(bb.instructions):
            if inst.name in made:
                continue
            si = inst.sync_info
            waits = list(si.on_wait) if si and si.on_wait else []
            if len(waits) <= limit:
                continue
            si.on_wait = waits[:limit]
            nops = []
            for w in waits[limit:]:
                ni = nc.engines[inst.engine].nop(nofuse=True, hint="wait_split")
                ni.ins.sync_info = mybir.SyncInfo(on_wait=[w], on_update=[])
                nops.append(ni.ins)
                made.add(ni.ins.name)
            pre[(bb.name, inst.name)] = nops
            n_split += len(nops)
    for bb in blocks:
        out = []
        for inst in list(bb.instructions):
            if inst.name in made:
                continue
            out.extend(pre.get((bb.name, inst.name), []))
            out.append(inst)
        bb.instructions = out
    return n_split


def _ap3(base, d1, d2, s1, s2):
    return bass.AP(tensor=base.tensor, offset=base.offset,
                   ap=[base.ap[0], [s1, d1], [s2, d2]])


def _wrap_idx(vals, nb):
    out = np.zeros((128, nb * 128), np.int16)
    for b in range(nb):
        seg = vals[b * EB:(b + 1) * EB].reshape(128, 16).T
        for g in range(8):
            out[g * 16:(g + 1) * 16, b * 128:(b + 1) * 128] = seg
    return out


def _preprocess(edge_index):
    """Partition edges by dst core/tile with a chunk schedule common to all
    cores; emit src-idx tables and dense one-hot [G01 | G01T] chunks."""
    src = np.concatenate([edge_index[0], np.arange(N, dtype=np.int64)])
    dst = np.concatenate([edge_index[1], np.arange(N, dtype=np.int64)])
    raw = []
    for c in range(NCORES):
        m = (dst >= c * PN) & (dst < (c + 1) * PN)
        s, dl = src[m], dst[m] - c * PN
        tid = dl // 128
        raw.append([(s[tid == t], dl[tid == t] - t * 128) for t in range(NTL)])
    kt = [max((len(raw[c][t][0]) + CHUNK - 1) // CHUNK for c in range(NCORES))
          for t in range(NTL)]
    nch = sum(kt)
    nch_p = (nch + CPB - 1) // CPB * CPB
    nb = nch_p // CPB
    ct = sum(([t] * kt[t] for t in range(NTL)), []) + [NTL - 1] * (nch_p - nch)
    start = [i == 0 or ct[i] != ct[i - 1] for i in range(nch_p)]
    stop = [i == nch_p - 1 or ct[i + 1] != ct[i] for i in range(nch_p)]
    per_core = []
    for c in range(NCORES):
        s_l, d_l = [], []
        for t in range(NTL):
            st, dt_ = raw[c][t]
            pad = kt[t] * CHUNK - len(st)
            s_l.append(np.concatenate([st, np.full(pad, N, np.int64)]))
            d_l.append(np.concatenate([dt_, -np.ones(pad, np.int64)]))
        pad = (nch_p - nch) * CHUNK
        s_ = np.concatenate(s_l + [np.full(pad, N, np.int64)])
        d_ = np.concatenate(d_l + [-np.ones(pad, np.int64)])
        g01 = np.zeros((nch_p, 128, 256), ml_dtypes.bfloat16)
        ei = np.arange(nch_p * 128)
        real = d_ >= 0
        ch, ep, dp = ei[real] // 128, ei[real] % 128, d_[real].astype(np.int64)
        g01[ch, ep, dp] = 1
        g01[ch, dp, 128 + ep] = 1
        per_core.append({
            "idx_main": _wrap_idx(s_.astype(np.int16), nb),
            "g01cat": g01,
        })
    return per_core, nb, ct, start, stop


def build(nb, ct, start, stop):
    nch = nb * CPB
    mode, NQ, PF = CFG["mode"], CFG["nq"], CFG["pf"]
    nc = bass.Bass(num_devices=NCORES, num_swdge_queues=NQ)

    xt_in = nc.declare_dram_parameter("xT", [F_IN, N], F32, isOutput=False)
    w0p_in = nc.declare_dram_parameter("W0perm", [F_IN, 512], F32, isOutput=False)
    w0pt_in = nc.declare_dram_parameter("W0permT", [512, F_IN], F32, isOutput=False)
    ac0_in = nc.declare_dram_parameter("Acat0p", [512, 16], F32, isOutput=False)
    w1p_in = nc.declare_dram_parameter("W1perm", [512, C], F32, isOutput=False)
    w1pt_in = nc.declare_dram_parameter("W1permT", [C, 512], F32, isOutput=False)
    ac1_in = nc.declare_dram_parameter("Acat1", [C, 2], F32, isOutput=False)
    wct_in = nc.declare_dram_parameter("WcT", [1, C], F32, isOutput=False)
    im_in = nc.declare_dram_parameter("idx_main", [128, nb * 128], I16, isOutput=False)
    g01_in = nc.declare_dram_parameter("g01cat", [nch, 128, 256], BF16, isOutput=False)
    out_fin = nc.declare_dram_parameter("out", [1, 1], F32, isOutput=True)
    dbg_hrow = nc.declare_dram_parameter("dbg_hrow", [128, ROW0], BF16, isOutput=True)
    dbg_adt = nc.declare_dram_parameter("dbg_adt", [128, NTL * 8], BF16, isOutput=True)
    dbg_h1own = nc.declare_dram_parameter("dbg_h1own", [128, ROW1], BF16, isOutput=True)
    dbg_ck = nc.declare_dram_parameter("dbg_ck", [128, 48], F32, isOutput=True)

    hcat0 = nc.dram_tensor("hcat0", [HROWS, ROW0], BF16)
    h1own = nc.dram_tensor("h1own", [NTL * 128, ROW1], BF16)
    h1tab = nc.dram_tensor("h1tab", [N + 8, ROW1], BF16, addr_space="Shared")
    pool_src = nc.dram_tensor("pool_src", [1, C], F32)
    pool_red = nc.dram_tensor("pool_red", [1, C], F32, addr_space="Shared")

    nc.gpsimd.load_library(library_config.mlp)
    eb_reg = nc.gpsimd.to_reg(EB)

    hc_all = bass.AP(tensor=hcat0[:, :].tensor, offset=0,
                     ap=[[ROW0, N + 1], [1, ROW0]])
    h1_all = bass.AP(tensor=h1tab[:, :].tensor, offset=0,
                     ap=[[ROW1, N + 1], [1, ROW1]])

    with FixedTileContext(nc) as tc:
        with tc.tile_pool(name="sg", bufs=1) as sg, \
             tc.tile_pool(name="op", bufs=3) as op, \
             tc.tile_pool(name="wk", bufs=3) as wk, \
             tc.tile_pool(name="ep", bufs=2) as ep:

            psW_cm = tc.tile_pool(name="psW", bufs=2, space="PSUM")
            psW = psW_cm.__enter__()
            # ---------- weights ----------
            ident = sg.tile([128, 128], BF16)
            make_identity(nc, ident[:])
            eps_sb = sg.tile([128, 1], F32)
            nc.vector.memset(eps_sb[:], 1e-20)
            m1_sb = sg.tile([128, 1], F32)
            nc.vector.memset(m1_sb[:], -1.0)
            ones_sb = sg.tile([128, 1], BF16)
            nc.vector.memset(ones_sb[:], 1.0)
            wct_sb = sg.tile([1, C], F32)
            nc.sync.dma_start(out=wct_sb[:], in_=wct_in[:])
            idxm_sb = sg.tile([128, nb * 128], I16)
            nc.sync.dma_start(out=idxm_sb[:], in_=im_in[:])

            w0cat = sg.tile([128, 528], BF16)
            nc.gpsimd.dma_start(out=w0cat[:, 0:512], in_=w0p_in[:])
            w0pt = sg.tile([128, 4, 128], BF16)
            nc.gpsimd.dma_start(
                out=w0pt[:],
                in_=bass.AP(tensor=w0pt_in[:, :].tensor, offset=0,
                            ap=[[128, 128], [128 * 128, 4], [1, 128]]))
            ac0 = sg.tile([128, 4, 16], BF16)
            nc.gpsimd.dma_start(
                out=ac0[:],
                in_=bass.AP(tensor=ac0_in[:, :].tensor, offset=0,
                            ap=[[16, 128], [16 * 128, 4], [1, 16]]))
            wext = psW.tile([128, 16], F32, space="PSUM", tag="wx")
            for q in range(4):
                nc.tensor.matmul(out=wext[:], lhsT=w0pt[:, q, :], rhs=ac0[:, q, :],
                                 start=(q == 0), stop=(q == 3))
            nc.vector.tensor_copy(out=w0cat[:, 512:528], in_=wext[:])

            w1cat = sg.tile([128, 4, C + 2], BF16)
            nc.gpsimd.dma_start(
                out=bass.AP(tensor=w1cat[:].tensor, offset=w1cat[:].offset,
                            ap=[w1cat[:].ap[0], [C + 2, 4], [1, C]]),
                in_=bass.AP(tensor=w1p_in[:, :].tensor, offset=0,
                            ap=[[C, 128], [C * 128, 4], [1, C]]))
            w1pt = sg.tile([C, 4, 128], BF16)
            nc.gpsimd.dma_start(
                out=w1pt[:],
                in_=bass.AP(tensor=w1pt_in[:, :].tensor, offset=0,
                            ap=[[512, C], [128, 4], [1, 128]]))
            ac1 = sg.tile([C, 2], BF16)
            nc.gpsimd.dma_start(out=ac1[:], in_=ac1_in[:])
            for q in range(4):
                w1e = psW.tile([128, 2], F32, space="PSUM", tag="wx")
                nc.tensor.matmul(out=w1e[:], lhsT=w1pt[:, q, :], rhs=ac1[:],
                                 start=True, stop=True)
                nc.vector.tensor_copy(out=w1cat[:, q, C:C + 2], in_=w1e[:])

            # ---------- phase A: hcat0 table (replicated) ----------
            xp_cm = tc.tile_pool(name="xp", bufs=1)
            xp = xp_cm.__enter__()
            xt_sb = xp.tile([128, NT0 * 128], BF16)
            for xq in range(4):
                nc.gpsimd.dma_start(out=xt_sb[:, xq * 5000:(xq + 1) * 5000],
                                    in_=xt_in[:, xq * 5000:(xq + 1) * 5000])
            nc.vector.memset(xt_sb[:, N:], 0.0)
            zrow = sg.tile([1, ROW0], BF16)
            nc.vector.memset(zrow[:], 0.0)
            nc.sync.dma_start(out=hcat0[N:N + 1, :], in_=zrow[:])

            for g4 in range((NT0 + 3) // 4):
                ts = list(range(g4 * 4, min(g4 * 4 + 4, NT0)))
                k4 = len(ts)
                stg = ep.tile([128, 4, 528], BF16, tag="stg")
                for k, t in enumerate(ts):
                    hp = psW.tile([128, 512], F32, space="PSUM", tag="hp")
                    epi = psW.tile([128, 16], F32, space="PSUM", tag="ep16")
                    nc.tensor.matmul(out=hp[:], lhsT=xt_sb[:, t * 128:(t + 1) * 128],
                                     rhs=w0cat[:, 0:512], start=True, stop=True)
                    nc.tensor.matmul(out=epi[:], lhsT=xt_sb[:, t * 128:(t + 1) * 128],
                                     rhs=w0cat[:, 512:528], start=True, stop=True)
                    if k % 2 == 0:
                        nc.vector.tensor_copy(out=stg[:, k, 0:512], in_=hp[:])
                        nc.scalar.activation(out=stg[:, k, 512:528], in_=epi[:],
                                             func=AF.Copy)
                    else:
                        nc.scalar.activation(out=stg[:, k, 0:512], in_=hp[:],
                                             func=AF.Copy)
                        nc.vector.tensor_copy(out=stg[:, k, 512:528], in_=epi[:])
                r0 = g4 * 4 * 128
                nc.sync.dma_start(
                    out=bass.AP(tensor=hcat0[:, :].tensor, offset=r0 * ROW0,
                                ap=[[ROW0, 128], [128 * ROW0, k4], [1, 528]]),
                    in_=stg[:, 0:k4, :])

            xp_cm.__exit__(None, None, None)

            # ---------- per-core ad table readback ----------
            pid = nc.sync.partition_id()
            adt = sg.tile([128, NTL, 8], BF16)
            nc.sync.dma_start(
                out=adt[:],
                in_=bass.AP(tensor=hcat0[:, :].tensor,
                            offset=pid * (PN * ROW0) + 520,
                            ap=[[ROW0, 128], [128 * ROW0, NTL], [1, 8]]))

            psW_cm.__exit__(None, None, None)
            psA_cm = tc.tile_pool(name="psA", bufs=2, space="PSUM")
            psA = psA_cm.__enter__()
            psB_cm = tc.tile_pool(name="psB", bufs=1, space="PSUM")
            psB = psB_cm.__enter__()
            psT_cm = tc.tile_pool(name="psT", bufs=2, space="PSUM")
            psT = psT_cm.__enter__()

            nc.sync.dma_start(out=dbg_hrow[:, :], in_=hcat0[0:128, :])
            nc.sync.dma_start(
                out=dbg_adt[:, :],
                in_=bass.AP(tensor=adt[:].tensor, offset=adt[:].offset,
                            ap=[adt[:].ap[0], [1, NTL * 8]]))

            # ---------- phase B: layer-0 aggregation + h1 rows ----------
            gp_cm = tc.tile_pool(name="gp", bufs=3)
            gp = gp_cm.__enter__()
            dsem = [nc.alloc_semaphore(f"dsB{q}") for q in range(NQ)]

            def issue_gather(b, pool_tag, elem, table_ap, sems):
                g = gp.tile([128, CPB, elem], BF16, tag=pool_tag)
                q = b % NQ
                if mode == "prep":
                    nc.gpsimd.dma_gather(
                        out_ap=g[:], in_ap=table_ap,
                        idxs_ap=idxm_sb[:, b * 128:(b + 1) * 128],
                        num_idxs=EB, num_idxs_reg=eb_reg, elem_size=elem,
                        single_packet=False, prepare_only=True, sem=sems[q],
                        queue_num=q)
                    nc.gpsimd.trigger_dma(count=None, queue_num=q)
                else:
                    nc.gpsimd.dma_gather(
                        out_ap=g[:], in_ap=table_ap,
                        idxs_ap=idxm_sb[:, b * 128:(b + 1) * 128],
                        num_idxs=EB, num_idxs_reg=eb_reg, elem_size=elem,
                        single_packet=False, queue_num=q)
                return g

            def load_g01(b):
                t = op.tile([128, CPB, 256], BF16, tag="g01")
                nc.sync.dma_start(
                    out=t[:],
                    in_=bass.AP(tensor=g01_in[:, :, :].tensor, offset=b * CPB * 128 * 256,
                                ap=[[256, 128], [128 * 256, CPB], [1, 256]]))
                return t

            g0_tiles = {b: issue_gather(b, "g0", ROW0, hc_all, dsem) for b in range(min(PF, nb))}
            g01_tiles = {b: load_g01(b) for b in range(min(PF, nb))}
            aggp = zp = None
            for b in range(nb):
                if b + PF < nb:
                    g0_tiles[b + PF] = issue_gather(b + PF, "g0", ROW0, hc_all, dsem)
                    g01_tiles[b + PF] = load_g01(b + PF)
                g0 = g0_tiles.pop(b)
                go = g01_tiles.pop(b)
                for cpos in range(CPB):
                    i = b * CPB + cpos
                    t = ct[i]
                    if start[i]:
                        aggp = psA.tile([128, 512], F32, space="PSUM", tag="agg")
                        zp = psB.tile([128, 8], F32, space="PSUM", tag="z")
                    adp = psB.tile([128, 8], F32, space="PSUM", tag="adp")
                    nc.tensor.matmul(out=adp[:], lhsT=go[:, cpos, 128:256],
                                     rhs=adt[:, t, :], start=True, stop=True)
                    e0 = wk.tile([128, 8], F32, tag="e0")
                    nc.vector.tensor_tensor(out=e0[:], in0=adp[:],
                                            in1=g0[:, cpos, 512:520], op=ALU.add)
                    lr = wk.tile([128, 8], F32, tag="lr")
                    nc.scalar.activation(out=lr[:], in_=e0[:], func=AF.Prelu,
                                         alpha=NEG_SLOPE)
                    pbf = wk.tile([128, 8], BF16, tag="pbf")
                    nc.scalar.activation(out=pbf[:], in_=lr[:], func=AF.Exp)
                    if i == 0:
                        ckb = wk.tile([128, 48], F32, tag="ckb")
                        nc.vector.tensor_copy(out=ckb[:, 0:8], in_=adp[:])
                        nc.vector.tensor_copy(out=ckb[:, 8:16], in_=e0[:])
                        nc.vector.tensor_copy(out=ckb[:, 16:24], in_=lr[:])
                        nc.vector.tensor_copy(out=ckb[:, 24:32], in_=pbf[:])
                        nc.vector.tensor_copy(out=ckb[:, 32:40], in_=g0[:, cpos, 512:520])
                        nc.vector.tensor_copy(out=ckb[:, 40:48], in_=g0[:, cpos, 0:8])
                        nc.sync.dma_start(out=dbg_ck[:, :], in_=ckb[:])
                    msg = wk.tile([128, 512], BF16, tag="msg")
                    nc.vector.tensor_tensor(out=msg[:], in0=g0[:, cpos, 0:512],
                                            in1=_ap3(pbf[:], 64, 8, 0, 1), op=ALU.mult)
                    nc.tensor.matmul(out=aggp[:], lhsT=go[:, cpos, 0:128], rhs=msg[:],
                                     start=start[i], stop=stop[i])
                    nc.tensor.matmul(out=zp[:], lhsT=go[:, cpos, 0:128], rhs=pbf[:],
                                     start=start[i], stop=stop[i])
                    if stop[i]:
                        rows = min(128, PN - t * 128)
                        zli = ep.tile([128, 8], F32, tag="zli")
                        nc.scalar.activation(out=zli[:], in_=zp[:], func=AF.Ln, bias=eps_sb[:])
                        zinv = ep.tile([128, 8], BF16, tag="zinv")
                        nc.scalar.activation(out=zinv[:], in_=zli[:], func=AF.Exp,
                                             scale=-1.0)
                        u = ep.tile([128, 512], BF16, tag="u")
                        nc.vector.tensor_tensor(out=u[:], in0=aggp[:],
                                                in1=_ap3(zinv[:], 64, 8, 0, 1),
                                                op=ALU.mult)
                        t1 = ep.tile([128, 512], BF16, tag="t1")
                        nc.scalar.activation(out=t1[:], in_=u[:], func=AF.Relu,
                                             scale=-1.0)
                        t2 = ep.tile([128, 512], BF16, tag="t2")
                        nc.scalar.activation(out=t2[:], in_=t1[:], func=AF.Exp,
                                             scale=-1.0)
                        t3 = ep.tile([128, 512], BF16, tag="t3")
                        nc.scalar.activation(out=t3[:], in_=t2[:], func=AF.Prelu,
                                             bias=m1_sb[:], alpha=1.0)
                        h1in = ep.tile([128, 512], BF16, tag="h1in")
                        nc.vector.tensor_tensor(out=h1in[:], in0=u[:], in1=t3[:],
                                                op=ALU.max)
                        h1t = ep.tile([128, 4, 128], BF16, tag="h1t")
                        for q in range(4):
                            tp = psT.tile([128, 128], BF16, space="PSUM", tag="tp")
                            nc.tensor.transpose(out=tp[:, :rows],
                                                in_=h1in[:rows, q * 128:(q + 1) * 128],
                                                identity=ident[:rows, :rows])
                            nc.scalar.activation(out=h1t[:, q, :rows], in_=tp[:, :rows],
                                                 func=AF.Copy)
                        h1ps = psB.tile([128, C + 2], F32, space="PSUM", tag="h1")
                        for q in range(4):
                            nc.tensor.matmul(out=h1ps[:rows, :], lhsT=h1t[:, q, :rows],
                                             rhs=w1cat[:, q, :], start=(q == 0),
                                             stop=(q == 3))
                        h1row = ep.tile([128, ROW1], BF16, tag="h1row")
                        nc.vector.memset(h1row[:, C + 2:ROW1], 0.0)
                        nc.vector.tensor_copy(out=h1row[:rows, 0:C + 2],
                                              in_=h1ps[:rows, :])
                        nc.sync.dma_start(out=h1own[t * 128:t * 128 + rows, :],
                                          in_=h1row[:rows, :])

            gp_cm.__exit__(None, None, None)
            nc.sync.dma_start(out=dbg_h1own[:, :], in_=h1own[0:128, :])

            # ---------- phase C: AllGather + ad1 readback ----------
            zrow1 = sg.tile([8, ROW1], BF16)
            nc.vector.memset(zrow1[:], 0.0)
            nc.sync.dma_start(out=h1tab[N:N + 8, :], in_=zrow1[:])
            nc.gpsimd.collective_compute(
                "AllGather", ALU.bypass, replica_groups=[list(range(NCORES))],
                ins=[h1own[0:PN, :]], outs=[h1tab[0:N, :]])
            ad1t = sg.tile([128, NTL, 1], BF16)
            nc.sync.dma_start(
                out=ad1t[:],
                in_=bass.AP(tensor=h1own[:, :].tensor, offset=C + 1,
                            ap=[[ROW1, 128], [128 * ROW1, NTL], [1, 1]]))

            # ---------- phase D: layer-1 aggregation + pooling ----------
            d_ap = CFG["d_ap"]
            if d_ap:
                nc.gpsimd.load_library(library_config.ap_gather)
            pool_acc = sg.tile([1, C], F32)
            nc.vector.memset(pool_acc[:], 0.0)
            with tc.tile_pool(name="dp", bufs=1) as dp, \
                 tc.tile_pool(name="dg", bufs=3) as dg:
                if d_ap:
                    tab1 = dp.tile([128, 20000, 2], BF16)
                    nc.sync.dma_start(
                        out=bass.AP(tensor=tab1[:].tensor, offset=tab1[:].offset,
                                    ap=[tab1[:].ap[0], [2, 20000], [1, 1]]),
                        in_=h1tab[0:N, :], transpose=True)

                def issue_apg(b):
                    if not d_ap:
                        g = dg.tile([128, CPB, ROW1], BF16, tag="g1t")
                        nc.gpsimd.dma_gather(
                            out_ap=g[:], in_ap=h1_all,
                            idxs_ap=idxm_sb[:, b * 128:(b + 1) * 128],
                            num_idxs=EB, num_idxs_reg=eb_reg, elem_size=ROW1,
                            single_packet=False, queue_num=b % NQ)
                        return g
                    g = dg.tile([128, EB, 2], BF16, tag="g1t")
                    nc.gpsimd.ap_gather(
                        out_ap=g[:], in_ap=tab1[:],
                        idxs_ap=idxm_sb[:, b * 128:(b + 1) * 128],
                        channels=128, num_elems=20000, d=2, num_idxs=EB)
                    return g

                g1_tiles = {b: issue_apg(b) for b in range(min(PF, nb))}
                g01_tiles = {b: load_g01(b) for b in range(min(PF, nb))}
                for b in range(nb):
                    if b + PF < nb:
                        g1_tiles[b + PF] = issue_apg(b + PF)
                        g01_tiles[b + PF] = load_g01(b + PF)
                    g1 = g1_tiles.pop(b)
                    go = g01_tiles.pop(b)
                    for cpos in range(CPB):
                        i = b * CPB + cpos
                        t = ct[i]
                        if start[i]:
                            aggp = psA.tile([128, C], F32, space="PSUM", tag="agg")
                            zp = psB.tile([128, 1], F32, space="PSUM", tag="z")
                        if d_ap:
                            tp1 = psT.tile([128, 128], BF16, space="PSUM", tag="tp")
                            nc.tensor.transpose(
                                out=tp1[:],
                                in_=g1[:, cpos * 128:(cpos + 1) * 128, 0:1],
                                identity=ident[:])
                            h1src = tp1
                            as1v = tp1[:, C:C + 1]
                        else:
                            h1src = None
                            as1v = g1[:, cpos, C:C + 1]
                        as1s = wk.tile([128, 1], BF16, tag="as1")
                        nc.vector.tensor_copy(out=as1s[:], in_=as1v)
                        adp = psB.tile([128, 1], F32, space="PSUM", tag="adp")
                        nc.tensor.matmul(out=adp[:], lhsT=go[:, cpos, 128:256],
                                         rhs=ad1t[:, t, :], start=True, stop=True)
                        e0 = wk.tile([128, 1], F32, tag="e1")
                        nc.vector.tensor_tensor(out=e0[:], in0=adp[:],
                                                in1=as1s[:], op=ALU.add)
                        lr = wk.tile([128, 1], F32, tag="lr1")
                        nc.scalar.activation(out=lr[:], in_=e0[:], func=AF.Prelu,
                                             alpha=NEG_SLOPE)
                        pbf = wk.tile([128, 1], BF16, tag="pbf1")
                        nc.scalar.activation(out=pbf[:], in_=lr[:], func=AF.Exp)
                        msg = wk.tile([128, C], BF16, tag="msg1")
                        m_in0 = h1src[:, 0:C] if d_ap else g1[:, cpos, 0:C]
                        nc.vector.tensor_tensor(out=msg[:], in0=m_in0,
                                                in1=_ap3(pbf[:], 1, C, 1, 0),
                                                op=ALU.mult)
                        nc.tensor.matmul(out=aggp[:], lhsT=go[:, cpos, 0:128], rhs=msg[:],
                                         start=start[i], stop=stop[i])
                        nc.tensor.matmul(out=zp[:], lhsT=go[:, cpos, 0:128], rhs=pbf[:],
                                         start=start[i], stop=stop[i])
                        if stop[i]:
                            rows = min(128, PN - t * 128)
                            zli = ep.tile([128, 1], F32, tag="zl1")
                            nc.scalar.activation(out=zli[:], in_=zp[:], func=AF.Ln,
                                                 bias=eps_sb[:])
                            zinv = ep.tile([128, 1], BF16, tag="zi1")
                            nc.scalar.activation(out=zinv[:], in_=zli[:], func=AF.Exp,
                                                 scale=-1.0)
                            o1 = ep.tile([128, C], BF16, tag="o1")
                            nc.vector.tensor_tensor(out=o1[:], in0=aggp[:],
                                                    in1=_ap3(zinv[:], 1, C, 1, 0),
                                                    op=ALU.mult)
                            pps = psB.tile([1, C], F32, space="PSUM", tag="pool")
                            nc.tensor.matmul(out=pps[:], lhsT=ones_sb[:rows, :],
                                             rhs=o1[:rows, :], start=True, stop=True)
                            nc.vector.tensor_tensor(out=pool_acc[:], in0=pool_acc[:],
                                                    in1=pps[:], op=ALU.add)

            # ---------- final: AllReduce, logit, sigmoid ----------
            nc.sync.dma_start(out=pool_src[:, :], in_=pool_acc[:])
            nc.gpsimd.collective_compute(
                "AllReduce", ALU.add, replica_groups=[list(range(NCORES))],
                ins=[pool_src[:, :]], outs=[pool_red[:, :]])
            pr = sg.tile([1, C], F32)
            nc.sync.dma_start(out=pr[:], in_=pool_red[:, :])
            tmul = sg.tile([1, C], F32)
            nc.vector.tensor_tensor(out=tmul[:], in0=pr[:], in1=wct_sb[:], op=ALU.mult)
            sres = sg.tile([1, 1], F32)
            nc.vector.tensor_reduce(out=sres[:], in_=tmul[:], axis=mybir.AxisListType.X,
                                    op=ALU.add)
            nc.vector.tensor_scalar(out=sres[:], in0=sres[:], scalar1=-1.0 / N,
                                    scalar2=None, op0=ALU.mult)
            nc.scalar.activation(out=sres[:], in_=sres[:], func=AF.Exp)
            nc.vector.tensor_scalar(out=sres[:], in0=sres[:], scalar1=1.0,
                                    scalar2=None, op0=ALU.add)
            nc.vector.reciprocal(out=sres[:], in_=sres[:])
            nc.sync.dma_start(out=out_fin[:, :], in_=sres[:])
            psT_cm.__exit__(None, None, None)
            psB_cm.__exit__(None, None, None)
            psA_cm.__exit__(None, None, None)

    ns = _legalize_multi_waits(nc)
    print(f"[kernel_new] split {ns} excess sem waits onto nops")
    nc.finalize()
    lower_extended_insts(nc)
    return nc


def _host_inputs(x, W0, W1, a_src0, a_dst0, a_src1, a_dst1, Wc):
    """Layout-only input transforms. Layer-0 features are permuted to
    interleaved order f' = c*8 + h (h innermost)."""
    perm0 = np.empty(512, np.int64)            # perm0[f'] = original col
    for h in range(H):
        for c in range(C):
            perm0[c * H + h] = h * C + c
    W0perm = np.ascontiguousarray(x.dtype.type(0) + W0[:, perm0], np.float32)
    acat0p = np.zeros((512, 16), np.float32)
    for h in range(H):
        for c in range(C):
            acat0p[c * H + h, h] = a_src0[h, c]
            acat0p[c * H + h, 8 + h] = a_dst0[h, c]
    W1perm = np.ascontiguousarray(W1[perm0, :], np.float32)
    acat1 = np.zeros((C, 2), np.float32)
    acat1[:, 0] = a_src1[0]
    acat1[:, 1] = a_dst1[0]
    return {
        "xT": np.ascontiguousarray(x.T, np.float32),
        "W0perm": W0perm,
        "W0permT": np.ascontiguousarray(W0perm.T, np.float32),
        "Acat0p": acat0p,
        "W1perm": W1perm,
        "W1permT": np.ascontiguousarray(W1perm.T, np.float32),
        "Acat1": acat1,
        "WcT": np.ascontiguousarray(Wc.reshape(1, C), np.float32),
    }


_RUN_KW = {}
LAST = {}


def kernel(x, edge_index, W0, a_src0, a_dst0, b0, W1, a_src1, a_dst1, b1, Wc, bc):
    x = np.asarray(x)
    edge_index = np.asarray(edge_index)
    per_core, nb, ct, start, stop = _preprocess(edge_index.astype(np.int64))
    nc = build(nb, ct, start, stop)
    shared = _host_inputs(x, np.asarray(W0), np.asarray(W1),
                          np.asarray(a_src0), np.asarray(a_dst0),
                          np.asarray(a_src1), np.asarray(a_dst1), np.asarray(Wc))
    in_maps = [{**shared, **per_core[c]} for c in range(NCORES)]
    res = run_bass_kernel_spmd(nc, in_maps, list(range(NCORES)), **_RUN_KW)
    LAST["res"] = res
    return np.asarray(res.results[0]["out"]).reshape(-1).astype(np.float32)


# revision 3
# speedup vs baseline: 1.0064x; 1.0064x over previous
"""Two-layer GAT on 8 TRN2 cores — redesigned for gather/vector throughput.

Key changes vs baseline kernel.py:
- One merged row gather per edge per layer ([h|as] in-row); ad via one-hot
  matmul from an SBUF table of the core's own dst nodes (no ad gathers).
- One-hot matrices G01/G01T precomputed on host (index-only data), streamed
  densely via HWDGE — no per-chunk is_equal on DVE.
- Host passes xT/W0perm/W1perm layouts so phase A needs no PE transposes;
  phase-A PSUM results are written straight to DRAM with cast DMAs in
  4-tile batches.
- Leaky-relu/exp/1/z moved to the ACT engine (Lrelu/Exp/Ln); msg multiply is
  the only big DVE op per chunk.
- Layer-0 feature order is interleaved (f' = c*8 + h) so the per-edge alpha
  broadcast has unit inner stride.
- Gathers use prepare_only+trigger_dma on rotating SWDGE queues (cfg).
"""

import numpy as np
import ml_dtypes

import concourse.bass as bass
import concourse.tile as tile
from concourse import mybir
from concourse.vector_clock import ScopedClock
from concourse.masks import make_identity
from concourse.bass_utils import run_bass_kernel_spmd
from concourse.library_overlay import lower_extended_insts
from concourse import library_config

F32 = mybir.dt.float32
BF16 = mybir.dt.bfloat16
I16 = mybir.dt.int16
AF = mybir.ActivationFunctionType
ALU = mybir.AluOpType

NCORES = 8
N, E, F_IN, C, H = 20000, 320000, 128, 64, 8
NEG_SLOPE = 0.2
PN = N // NCORES
CHUNK = 128
CPB = 16
EB = CHUNK * CPB
ROW0 = 640            # bf16: [h 512 | as 8 | ad 8 | pad] -> 1280 B rows
ROW1 = 128            # bf16: [h1 64 | as1 | ad1 | pad] -> 256 B rows
NT0 = (N + 127) // 128          # global node tiles (157)
NTL = (PN + 127) // 128         # local dst tiles per core (20)
HROWS = NT0 * 128 + 8           # hcat row allocation (full-tile writes)

CFG = {"mode": "normal", "nq": 4, "pf": 3, "d_ap": False}


class FixedTileContext(tile.TileContext):
    """Walrus here rejects >1 sem wait on the tail Drain: hoist onto NOPs."""

    def _drain_and_barrier(self, tick_clock, wait_clock):
        nop = self.nc.sync.nop(nofuse=True, hint="pre_drain_waits")
        wait_clock.add_sem_waits(nop.ins, ScopedClock({None: tick_clock.global_clock}))
        si = nop.ins.sync_info
        waits = list(si.on_wait) if si and si.on_wait else []
        if len(waits) > 1:
            si.on_wait = [waits[0]]
            for w in waits[1:]:
                n2 = self.nc.sync.nop(nofuse=True, hint="pre_drain_waits")
                n2.ins.sync_info = mybir.SyncInfo(on_wait=[w], on_update=[])
        self.nc.sync.drain()
        self.nc.all_engine_barrier()
        popped = self.nc._tile_sem_poison_stack.pop()
        assert popped is self._sem_poison
        self.nc.clear_and_free_semaphores(list(self.sems.allocated().values()))
        self.nc.all_engine_barrier()


def _legalize_multi_waits(nc, limit=1):
    n_split = 0
    pre = {}
    made = set()
    blocks = [bb for f in nc.m.functions for bb in f.blocks]
    for bb in blocks:
        for inst in list(bb.instructions):
            if inst.name in made:
                continue
            si = inst.sync_info
            waits = list(si.on_wait) if si and si.on_wait else []
            if len(waits) <= limit:
                continue
            si.on_wait = waits[:limit]
            nops = []
            for w in waits[limit:]:
                ni = nc.engines[inst.engine].nop(nofuse=True, hint="wait_split")
                ni.ins.sync_info = mybir.SyncInfo(on_wait=[w], on_update=[])
                nops.append(ni.ins)
                made.add(ni.ins.name)
            pre[(bb.name, inst.name)] = nops
            n_split += len(nops)
    for bb in blocks:
        out = []
        for inst in list(bb.instructions):
            if inst.name in made:
                continue
            out.extend(pre.get((bb.name, inst.name), []))
            out.append(inst)
        bb.instructions = out
    return n_split


def _ap3(base, d1, d2, s1, s2):
    return bass.AP(tensor=base.tensor, offset=base.offset,
                   ap=[base.ap[0], [s1, d1], [s2, d2]])


def _wrap_idx(vals, nb):
    out = np.zeros((128, nb * 128), np.int16)
    for b in range(nb):
        seg = vals[b * EB:(b + 1) * EB].reshape(128, 16).T
        for g in range(8):
            out[g * 16:(g + 1) * 16, b * 128:(b + 1) * 128] = seg
    return out


def _preprocess(edge_index):
    """Partition edges by dst core/tile with a chunk schedule common to all
    cores; emit src-idx tables and dense one-hot [G01 | G01T] chunks."""
    src = np.concatenate([edge_index[0], np.arange(N, dtype=np.int64)])
    dst = np.concatenate([edge_index[1], np.arange(N, dtype=np.int64)])
    raw = []
    for c in range(NCORES):
        m = (dst >= c * PN) & (dst < (c + 1) * PN)
        s, dl = src[m], dst[m] - c * PN
        tid = dl // 128
        raw.append([(s[tid == t], dl[tid == t] - t * 128) for t in range(NTL)])
    kt = [max((len(raw[c][t][0]) + CHUNK - 1) // CHUNK for c in range(NCORES))
          for t in range(NTL)]
    nch = sum(kt)
    nch_p = (nch + CPB - 1) // CPB * CPB
    nb = nch_p // CPB
    ct = sum(([t] * kt[t] for t in range(NTL)), []) + [NTL - 1] * (nch_p - nch)
    start = [i == 0 or ct[i] != ct[i - 1] for i in range(nch_p)]
    stop = [i == nch_p - 1 or ct[i + 1] != ct[i] for i in range(nch_p)]
    per_core = []
    for c in range(NCORES):
        s_l, d_l = [], []
        for t in range(NTL):
            st, dt_ = raw[c][t]
            pad = kt[t] * CHUNK - len(st)
            s_l.append(np.concatenate([st, np.full(pad, N, np.int64)]))
            d_l.append(np.concatenate([dt_, -np.ones(pad, np.int64)]))
        pad = (nch_p - nch) * CHUNK
        s_ = np.concatenate(s_l + [np.full(pad, N, np.int64)])
        d_ = np.concatenate(d_l + [-np.ones(pad, np.int64)])
        g01 = np.zeros((nch_p, 128, 256), ml_dtypes.bfloat16)
        ei = np.arange(nch_p * 128)
        real = d_ >= 0
        ch, ep, dp = ei[real] // 128, ei[real] % 128, d_[real].astype(np.int64)
        g01[ch, ep, dp] = 1
        g01[ch, dp, 128 + ep] = 1
        per_core.append({
            "idx_main": _wrap_idx(s_.astype(np.int16), nb),
            "g01cat": g01,
        })
    return per_core, nb, ct, start, stop


def build(nb, ct, start, stop):
    nch = nb * CPB
    mode, NQ, PF = CFG["mode"], CFG["nq"], CFG["pf"]
    nc = bass.Bass(num_devices=NCORES, num_swdge_queues=NQ)

    xt_in = nc.declare_dram_parameter("xT", [F_IN, N], F32, isOutput=False)
    w0p_in = nc.declare_dram_parameter("W0perm", [F_IN, 512], F32, isOutput=False)
    w0pt_in = nc.declare_dram_parameter("W0permT", [512, F_IN], F32, isOutput=False)
    ac0_in = nc.declare_dram_parameter("Acat0p", [512, 16], F32, isOutput=False)
    w1p_in = nc.declare_dram_parameter("W1perm", [512, C], F32, isOutput=False)
    w1pt_in = nc.declare_dram_parameter("W1permT", [C, 512], F32, isOutput=False)
    ac1_in = nc.declare_dram_parameter("Acat1", [C, 2], F32, isOutput=False)
    wct_in = nc.declare_dram_parameter("WcT", [1, C], F32, isOutput=False)
    im_in = nc.declare_dram_parameter("idx_main", [128, nb * 128], I16, isOutput=False)
    g01_in = nc.declare_dram_parameter("g01cat", [nch, 128, 256], BF16, isOutput=False)
    out_fin = nc.declare_dram_parameter("out", [1, 1], F32, isOutput=True)
    dbg_hrow = nc.declare_dram_parameter("dbg_hrow", [128, ROW0], BF16, isOutput=True)
    dbg_adt = nc.declare_dram_parameter("dbg_adt", [128, NTL * 8], BF16, isOutput=True)
    dbg_h1own = nc.declare_dram_parameter("dbg_h1own", [128, ROW1], BF16, isOutput=True)
    dbg_ck = nc.declare_dram_parameter("dbg_ck", [128, 48], F32, isOutput=True)

    hcat0 = nc.dram_tensor("hcat0", [HROWS, ROW0], BF16)
    h1own = nc.dram_tensor("h1own", [NTL * 128, ROW1], BF16)
    h1tab = nc.dram_tensor("h1tab", [N + 8, ROW1], BF16, addr_space="Shared")
    pool_src = nc.dram_tensor("pool_src", [1, C], F32)
    pool_red = nc.dram_tensor("pool_red", [1, C], F32, addr_space="Shared")

    nc.gpsimd.load_library(library_config.mlp)
    eb_reg = nc.gpsimd.to_reg(EB)

    hc_all = bass.AP(tensor=hcat0[:, :].tensor, offset=0,
                     ap=[[ROW0, N + 1], [1, ROW0]])
    h1_all = bass.AP(tensor=h1tab[:, :].tensor, offset=0,
                     ap=[[ROW1, N + 1], [1, ROW1]])

    with FixedTileContext(nc) as tc:
        with tc.tile_pool(name="sg", bufs=1) as sg, \
             tc.tile_pool(name="op", bufs=3) as op, \
             tc.tile_pool(name="wk", bufs=3) as wk, \
             tc.tile_pool(name="ep", bufs=2) as ep:

            psW_cm = tc.tile_pool(name="psW", bufs=2, space="PSUM")
            psW = psW_cm.__enter__()
            # ---------- weights ----------
            ident = sg.tile([128, 128], BF16)
            make_identity(nc, ident[:])
            eps_sb = sg.tile([128, 1], F32)
            nc.vector.memset(eps_sb[:], 1e-20)
            m1_sb = sg.tile([128, 1], F32)
            nc.vector.memset(m1_sb[:], -1.0)
            ones_sb = sg.tile([128, 1], BF16)
            nc.vector.memset(ones_sb[:], 1.0)
            wct_sb = sg.tile([1, C], F32)
            nc.sync.dma_start(out=wct_sb[:], in_=wct_in[:])
            idxm_sb = sg.tile([128, nb * 128], I16)
            nc.sync.dma_start(out=idxm_sb[:], in_=im_in[:])

            w0cat = sg.tile([128, 528], BF16)
            nc.gpsimd.dma_start(out=w0cat[:, 0:512], in_=w0p_in[:])
            w0pt = sg.tile([128, 4, 128], BF16)
            nc.gpsimd.dma_start(
                out=w0pt[:],
                in_=bass.AP(tensor=w0pt_in[:, :].tensor, offset=0,
                            ap=[[128, 128], [128 * 128, 4], [1, 128]]))
            ac0 = sg.tile([128, 4, 16], BF16)
            nc.gpsimd.dma_start(
                out=ac0[:],
                in_=bass.AP(tensor=ac0_in[:, :].tensor, offset=0,
                            ap=[[16, 128], [16 * 128, 4], [1, 16]]))
            wext = psW.tile([128, 16], F32, space="PSUM", tag="wx")
            for q in range(4):
                nc.tensor.matmul(out=wext[:], lhsT=w0pt[:, q, :], rhs=ac0[:, q, :],
                                 start=(q == 0), stop=(q == 3))
            nc.vector.tensor_copy(out=w0cat[:, 512:528], in_=wext[:])

            w1cat = sg.tile([128, 4, C + 2], BF16)
            nc.gpsimd.dma_start(
                out=bass.AP(tensor=w1cat[:].tensor, offset=w1cat[:].offset,
                            ap=[w1cat[:].ap[0], [C + 2, 4], [1, C]]),
                in_=bass.AP(tensor=w1p_in[:, :].tensor, offset=0,
                            ap=[[C, 128], [C * 128, 4], [1, C]]))
            w1pt = sg.tile([C, 4, 128], BF16)
            nc.gpsimd.dma_start(
                out=w1pt[:],
                in_=bass.AP(tensor=w1pt_in[:, :].tensor, offset=0,
                            ap=[[512, C], [128, 4], [1, 128]]))
            ac1 = sg.tile([C, 2], BF16)
            nc.gpsimd.dma_start(out=ac1[:], in_=ac1_in[:])
            for q in range(4):
                w1e = psW.tile([128, 2], F32, space="PSUM", tag="wx")
                nc.tensor.matmul(out=w1e[:], lhsT=w1pt[:, q, :], rhs=ac1[:],
                                 start=True, stop=True)
                nc.vector.tensor_copy(out=w1cat[:, q, C:C + 2], in_=w1e[:])

            # ---------- phase A: hcat0 table (replicated) ----------
            xp_cm = tc.tile_pool(name="xp", bufs=1)
            xp = xp_cm.__enter__()
            xt_sb = xp.tile([128, NT0 * 128], BF16)
            for xq in range(4):
                nc.gpsimd.dma_start(out=xt_sb[:, xq * 5000:(xq + 1) * 5000],
                                    in_=xt_in[:, xq * 5000:(xq + 1) * 5000])
            nc.vector.memset(xt_sb[:, N:], 0.0)
            zrow = sg.tile([1, ROW0], BF16)
            nc.vector.memset(zrow[:], 0.0)
            nc.sync.dma_start(out=hcat0[N:N + 1, :], in_=zrow[:])

            for g4 in range((NT0 + 3) // 4):
                ts = list(range(g4 * 4, min(g4 * 4 + 4, NT0)))
                k4 = len(ts)
                stg = ep.tile([128, 4, 528], BF16, tag="stg")
                for k, t in enumerate(ts):
                    hp = psW.tile([128, 512], F32, space="PSUM", tag="hp")
                    epi = psW.tile([128, 16], F32, space="PSUM", tag="ep16")
                    nc.tensor.matmul(out=hp[:], lhsT=xt_sb[:, t * 128:(t + 1) * 128],
                                     rhs=w0cat[:, 0:512], start=True, stop=True)
                    nc.tensor.matmul(out=epi[:], lhsT=xt_sb[:, t * 128:(t + 1) * 128],
                                     rhs=w0cat[:, 512:528], start=True, stop=True)
                    if k % 2 == 0:
                        nc.vector.tensor_copy(out=stg[:, k, 0:512], in_=hp[:])
                        nc.scalar.activation(out=stg[:, k, 512:528], in_=epi[:],
                                             func=AF.Copy)
                    else:
                        nc.scalar.activation(out=stg[:, k, 0:512], in_=hp[:],
                                             func=AF.Copy)
                        nc.vector.tensor_copy(out=stg[:, k, 512:528], in_=epi[:])
                r0 = g4 * 4 * 128
                nc.sync.dma_start(
                    out=bass.AP(tensor=hcat0[:, :].tensor, offset=r0 * ROW0,
                                ap=[[ROW0, 128], [128 * ROW0, k4], [1, 528]]),
                    in_=stg[:, 0:k4, :])

            xp_cm.__exit__(None, None, None)

            # ---------- per-core ad table readback ----------
            pid = nc.sync.partition_id()
            adt = sg.tile([128, NTL, 8], BF16)
            nc.sync.dma_start(
                out=adt[:],
                in_=bass.AP(tensor=hcat0[:, :].tensor,
                            offset=pid * (PN * ROW0) + 520,
                            ap=[[ROW0, 128], [128 * ROW0, NTL], [1, 8]]))

            psW_cm.__exit__(None, None, None)
            psA_cm = tc.tile_pool(name="psA", bufs=2, space="PSUM")
            psA = psA_cm.__enter__()
            psB_cm = tc.tile_pool(name="psB", bufs=1, space="PSUM")
            psB = psB_cm.__enter__()
            psT_cm = tc.tile_pool(name="psT", bufs=1, space="PSUM")
            psT = psT_cm.__enter__()
            psC_cm = tc.tile_pool(name="psC", bufs=2, space="PSUM")
            psC = psC_cm.__enter__()

            nc.sync.dma_start(out=dbg_hrow[:, :], in_=hcat0[0:128, :])
            nc.sync.dma_start(
                out=dbg_adt[:, :],
                in_=bass.AP(tensor=adt[:].tensor, offset=adt[:].offset,
                            ap=[adt[:].ap[0], [1, NTL * 8]]))

            # ---------- phase B: layer-0 aggregation + h1 rows ----------
            gp_cm = tc.tile_pool(name="gp", bufs=4)
            gp = gp_cm.__enter__()
            dsem = [nc.alloc_semaphore(f"dsB{q}") for q in range(NQ)]

            def issue_gather(b, pool_tag, elem, table_ap, sems):
                g = gp.tile([128, CPB, elem], BF16, tag=pool_tag)
                q = b % NQ
                if mode == "prep":
                    nc.gpsimd.dma_gather(
                        out_ap=g[:], in_ap=table_ap,
                        idxs_ap=idxm_sb[:, b * 128:(b + 1) * 128],
                        num_idxs=EB, num_idxs_reg=eb_reg, elem_size=elem,
                        single_packet=False, prepare_only=True, sem=sems[q],
                        queue_num=q)
                    nc.gpsimd.trigger_dma(count=None, queue_num=q)
                else:
                    nc.gpsimd.dma_gather(
                        out_ap=g[:], in_ap=table_ap,
                        idxs_ap=idxm_sb[:, b * 128:(b + 1) * 128],
                        num_idxs=EB, num_idxs_reg=eb_reg, elem_size=elem,
                        single_packet=False, queue_num=q)
                return g

            def load_g01(b):
                t = op.tile([128, CPB, 256], BF16, tag="g01")
                nc.sync.dma_start(
                    out=t[:],
                    in_=bass.AP(tensor=g01_in[:, :, :].tensor, offset=b * CPB * 128 * 256,
                                ap=[[256, 128], [128 * 256, CPB], [1, 256]]))
                return t

            g0_tiles = {b: issue_gather(b, "g0", ROW0, hc_all, dsem) for b in range(min(PF, nb))}
            g01_tiles = {b: load_g01(b) for b in range(min(PF, nb))}
            aggp = zp = None
            for b in range(nb):
                if b + PF < nb:
                    g0_tiles[b + PF] = issue_gather(b + PF, "g0", ROW0, hc_all, dsem)
                    g01_tiles[b + PF] = load_g01(b + PF)
                g0 = g0_tiles.pop(b)
                go = g01_tiles.pop(b)
                for cpos in range(CPB):
                    i = b * CPB + cpos
                    t = ct[i]
                    if start[i]:
                        aggp = psA.tile([128, 512], F32, space="PSUM", tag="agg")
                        zp = psB.tile([128, 8], F32, space="PSUM", tag="z")
                    adp = psC.tile([128, 8], F32, space="PSUM", tag="adp")
                    nc.tensor.matmul(out=adp[:], lhsT=go[:, cpos, 128:256],
                                     rhs=adt[:, t, :], start=True, stop=True)
                    e0 = wk.tile([128, 8], F32, tag="e0")
                    nc.vector.tensor_tensor(out=e0[:], in0=adp[:],
                                            in1=g0[:, cpos, 512:520], op=ALU.add)
                    lr = wk.tile([128, 8], F32, tag="lr")
                    nc.scalar.activation(out=lr[:], in_=e0[:], func=AF.Prelu,
                                         alpha=NEG_SLOPE)
                    pbf = wk.tile([128, 8], BF16, tag="pbf")
                    nc.scalar.activation(out=pbf[:], in_=lr[:], func=AF.Exp)
                    if i == 0:
                        ckb = wk.tile([128, 48], F32, tag="ckb")
                        nc.vector.tensor_copy(out=ckb[:, 0:8], in_=adp[:])
                        nc.vector.tensor_copy(out=ckb[:, 8:16], in_=e0[:])
                        nc.vector.tensor_copy(out=ckb[:, 16:24], in_=lr[:])
                        nc.vector.tensor_copy(out=ckb[:, 24:32], in_=pbf[:])
                        nc.vector.tensor_copy(out=ckb[:, 32:40], in_=g0[:, cpos, 512:520])
                        nc.vector.tensor_copy(out=ckb[:, 40:48], in_=g0[:, cpos, 0:8])
                        nc.sync.dma_start(out=dbg_ck[:, :], in_=ckb[:])
                    msg = wk.tile([128, 512], BF16, tag="msg")
                    nc.vector.tensor_tensor(out=msg[:], in0=g0[:, cpos, 0:512],
                                            in1=_ap3(pbf[:], 64, 8, 0, 1), op=ALU.mult)
                    nc.tensor.matmul(out=aggp[:], lhsT=go[:, cpos, 0:128], rhs=msg[:],
                                     start=start[i], stop=stop[i])
                    nc.tensor.matmul(out=zp[:], lhsT=go[:, cpos, 0:128], rhs=pbf[:],
                                     start=start[i], stop=stop[i])
                    if stop[i]:
                        rows = min(128, PN - t * 128)
                        zli = ep.tile([128, 8], F32, tag="zli")
                        nc.scalar.activation(out=zli[:], in_=zp[:], func=AF.Ln, bias=eps_sb[:])
                        zinv = ep.tile([128, 8], BF16, tag="zinv")
                        nc.scalar.activation(out=zinv[:], in_=zli[:], func=AF.Exp,
                                             scale=-1.0)
                        u = ep.tile([128, 512], BF16, tag="u")
                        nc.vector.tensor_tensor(out=u[:], in0=aggp[:],
                                                in1=_ap3(zinv[:], 64, 8, 0, 1),
                                                op=ALU.mult)
                        t1 = ep.tile([128, 512], BF16, tag="t1")
                        nc.scalar.activation(out=t1[:], in_=u[:], func=AF.Relu,
                                             scale=-1.0)
                        t2 = ep.tile([128, 512], BF16, tag="t2")
                        nc.scalar.activation(out=t2[:], in_=t1[:], func=AF.Exp,
                                             scale=-1.0)
                        t3 = ep.tile([128, 512], BF16, tag="t3")
                        nc.scalar.activation(out=t3[:], in_=t2[:], func=AF.Prelu,
                                             bias=m1_sb[:], alpha=1.0)
                        h1in = ep.tile([128, 512], BF16, tag="h1in")
                        nc.vector.tensor_tensor(out=h1in[:], in0=u[:], in1=t3[:],
                                                op=ALU.max)
                        h1t = ep.tile([128, 4, 128], BF16, tag="h1t")
                        for q in range(4):
                            tp = psT.tile([128, 128], BF16, space="PSUM", tag="tp")
                            nc.tensor.transpose(out=tp[:, :rows],
                                                in_=h1in[:rows, q * 128:(q + 1) * 128],
                                                identity=ident[:rows, :rows])
                            nc.scalar.activation(out=h1t[:, q, :rows], in_=tp[:, :rows],
                                                 func=AF.Copy)
                        h1ps = psB.tile([128, C + 2], F32, space="PSUM", tag="h1")
                        for q in range(4):
                            nc.tensor.matmul(out=h1ps[:rows, :], lhsT=h1t[:, q, :rows],
                                             rhs=w1cat[:, q, :], start=(q == 0),
                                             stop=(q == 3))
                        h1row = ep.tile([128, ROW1], BF16, tag="h1row")
                        nc.vector.memset(h1row[:, C + 2:ROW1], 0.0)
                        nc.vector.tensor_copy(out=h1row[:rows, 0:C + 2],
                                              in_=h1ps[:rows, :])
                        nc.sync.dma_start(out=h1own[t * 128:t * 128 + rows, :],
                                          in_=h1row[:rows, :])

            gp_cm.__exit__(None, None, None)
            nc.sync.dma_start(out=dbg_h1own[:, :], in_=h1own[0:128, :])

            # ---------- phase C: AllGather + ad1 readback ----------
            zrow1 = sg.tile([8, ROW1], BF16)
            nc.vector.memset(zrow1[:], 0.0)
            nc.sync.dma_start(out=h1tab[N:N + 8, :], in_=zrow1[:])
            nc.gpsimd.collective_compute(
                "AllGather", ALU.bypass, replica_groups=[list(range(NCORES))],
                ins=[h1own[0:PN, :]], outs=[h1tab[0:N, :]])
            ad1t = sg.tile([128, NTL, 1], BF16)
            nc.sync.dma_start(
                out=ad1t[:],
                in_=bass.AP(tensor=h1own[:, :].tensor, offset=C + 1,
                            ap=[[ROW1, 128], [128 * ROW1, NTL], [1, 1]]))

            # ---------- phase D: layer-1 aggregation + pooling ----------
            d_ap = CFG["d_ap"]
            if d_ap:
                nc.gpsimd.load_library(library_config.ap_gather)
            pool_acc = sg.tile([1, C], F32)
            nc.vector.memset(pool_acc[:], 0.0)
            with tc.tile_pool(name="dp", bufs=1) as dp, \
                 tc.tile_pool(name="dg", bufs=3) as dg:
                if d_ap:
                    tab1 = dp.tile([128, 20000, 2], BF16)
                    nc.sync.dma_start(
                        out=bass.AP(tensor=tab1[:].tensor, offset=tab1[:].offset,
                                    ap=[tab1[:].ap[0], [2, 20000], [1, 1]]),
                        in_=h1tab[0:N, :], transpose=True)

                def issue_apg(b):
                    if not d_ap:
                        g = dg.tile([128, CPB, ROW1], BF16, tag="g1t")
                        nc.gpsimd.dma_gather(
                            out_ap=g[:], in_ap=h1_all,
                            idxs_ap=idxm_sb[:, b * 128:(b + 1) * 128],
                            num_idxs=EB, num_idxs_reg=eb_reg, elem_size=ROW1,
                            single_packet=False, queue_num=b % NQ)
                        return g
                    g = dg.tile([128, EB, 2], BF16, tag="g1t")
                    nc.gpsimd.ap_gather(
                        out_ap=g[:], in_ap=tab1[:],
                        idxs_ap=idxm_sb[:, b * 128:(b + 1) * 128],
                        channels=128, num_elems=20000, d=2, num_idxs=EB)
                    return g

                g1_tiles = {b: issue_apg(b) for b in range(min(PF, nb))}
                g01_tiles = {b: load_g01(b) for b in range(min(PF, nb))}
                for b in range(nb):
                    if b + PF < nb:
                        g1_tiles[b + PF] = issue_apg(b + PF)
                        g01_tiles[b + PF] = load_g01(b + PF)
                    g1 = g1_tiles.pop(b)
                    go = g01_tiles.pop(b)
                    for cpos in range(CPB):
                        i = b * CPB + cpos
                        t = ct[i]
                        if start[i]:
                            aggp = psA.tile([128, C], F32, space="PSUM", tag="agg")
                            zp = psB.tile([128, 1], F32, space="PSUM", tag="z")
                        if d_ap:
                            tp1 = psT.tile([128, 128], BF16, space="PSUM", tag="tp")
                            nc.tensor.transpose(
                                out=tp1[:],
                                in_=g1[:, cpos * 128:(cpos + 1) * 128, 0:1],
                                identity=ident[:])
                            h1src = tp1
                            as1v = tp1[:, C:C + 1]
                        else:
                            h1src = None
                            as1v = g1[:, cpos, C:C + 1]
                        as1s = wk.tile([128, 1], BF16, tag="as1")
                        nc.vector.tensor_copy(out=as1s[:], in_=as1v)
                        adp = psC.tile([128, 1], F32, space="PSUM", tag="adp")
                        nc.tensor.matmul(out=adp[:], lhsT=go[:, cpos, 128:256],
                                         rhs=ad1t[:, t, :], start=True, stop=True)
                        e0 = wk.tile([128, 1], F32, tag="e1")
                        nc.vector.tensor_tensor(out=e0[:], in0=adp[:],
                                                in1=as1s[:], op=ALU.add)
                        lr = wk.tile([128, 1], F32, tag="lr1")
                        nc.scalar.activation(out=lr[:], in_=e0[:], func=AF.Prelu,
                                             alpha=NEG_SLOPE)
                        pbf = wk.tile([128, 1], BF16, tag="pbf1")
                        nc.scalar.activation(out=pbf[:], in_=lr[:], func=AF.Exp)
                        msg = wk.tile([128, C], BF16, tag="msg1")
                        m_in0 = h1src[:, 0:C] if d_ap else g1[:, cpos, 0:C]
                        nc.vector.tensor_tensor(out=msg[:], in0=m_in0,
                                                in1=_ap3(pbf[:], 1, C, 1, 0),
                                                op=ALU.mult)
                        nc.tensor.matmul(out=aggp[:], lhsT=go[:, cpos, 0:128], rhs=msg[:],
                                         start=start[i], stop=stop[i])
                        nc.tensor.matmul(out=zp[:], lhsT=go[:, cpos, 0:128], rhs=pbf[:],
                                         start=start[i], stop=stop[i])
                        if stop[i]:
                            rows = min(128, PN - t * 128)
                            zli = ep.tile([128, 1], F32, tag="zl1")
                            nc.scalar.activation(out=zli[:], in_=zp[:], func=AF.Ln,
                                                 bias=eps_sb[:])
                            zinv = ep.tile([128, 1], BF16, tag="zi1")
                            nc.scalar.activation(out=zinv[:], in_=zli[:], func=AF.Exp,
                                                 scale=-1.0)
                            o1 = ep.tile([128, C], BF16, tag="o1")
                            nc.vector.tensor_tensor(out=o1[:], in0=aggp[:],
                                                    in1=_ap3(zinv[:], 1, C, 1, 0),
                                                    op=ALU.mult)
                            pps = psB.tile([1, C], F32, space="PSUM", tag="pool")
                            nc.tensor.matmul(out=pps[:], lhsT=ones_sb[:rows, :],
                                             rhs=o1[:rows, :], start=True, stop=True)
                            nc.vector.tensor_tensor(out=pool_acc[:], in0=pool_acc[:],
                                                    in1=pps[:], op=ALU.add)

            # ---------- final: AllReduce, logit, sigmoid ----------
            nc.sync.dma_start(out=pool_src[:, :], in_=pool_acc[:])
            nc.gpsimd.collective_compute(
                "AllReduce", ALU.add, replica_groups=[list(range(NCORES))],
                ins=[pool_src[:, :]], outs=[pool_red[:, :]])
            pr = sg.tile([1, C], F32)
            nc.sync.dma_start(out=pr[:], in_=pool_red[:, :])
            tmul = sg.tile([1, C], F32)
            nc.vector.tensor_tensor(out=tmul[:], in0=pr[:], in1=wct_sb[:], op=ALU.mult)
            sres = sg.tile([1, 1], F32)
            nc.vector.tensor_reduce(out=sres[:], in_=tmul[:], axis=mybir.AxisListType.X,
                                    op=ALU.add)
            nc.vector.tensor_scalar(out=sres[:], in0=sres[:], scalar1=-1.0 / N,
                                    scalar2=None, op0=ALU.mult)
            nc.scalar.activation(out=sres[:], in_=sres[:], func=AF.Exp)
            nc.vector.tensor_scalar(out=sres[:], in0=sres[:], scalar1=1.0,
                                    scalar2=None, op0=ALU.add)
            nc.vector.reciprocal(out=sres[:], in_=sres[:])
            nc.sync.dma_start(out=out_fin[:, :], in_=sres[:])
            psC_cm.__exit__(None, None, None)
            psT_cm.__exit__(None, None, None)
            psB_cm.__exit__(None, None, None)
            psA_cm.__exit__(None, None, None)

    ns = _legalize_multi_waits(nc)
    print(f"[kernel_new] split {ns} excess sem waits onto nops")
    nc.finalize()
    lower_extended_insts(nc)
    return nc


def _host_inputs(x, W0, W1, a_src0, a_dst0, a_src1, a_dst1, Wc):
    """Layout-only input transforms. Layer-0 features are permuted to
    interleaved order f' = c*8 + h (h innermost)."""
    perm0 = np.empty(512, np.int64)            # perm0[f'] = original col
    for h in range(H):
        for c in range(C):
            perm0[c * H + h] = h * C + c
    W0perm = np.ascontiguousarray(x.dtype.type(0) + W0[:, perm0], np.float32)
    acat0p = np.zeros((512, 16), np.float32)
    for h in range(H):
        for c in range(C):
            acat0p[c * H + h, h] = a_src0[h, c]
            acat0p[c * H + h, 8 + h] = a_dst0[h, c]
    W1perm = np.ascontiguousarray(W1[perm0, :], np.float32)
    acat1 = np.zeros((C, 2), np.float32)
    acat1[:, 0] = a_src1[0]
    acat1[:, 1] = a_dst1[0]
    return {
        "xT": np.ascontiguousarray(x.T, np.float32),
        "W0perm": W0perm,
        "W0permT": np.ascontiguousarray(W0perm.T, np.float32),
        "Acat0p": acat0p,
        "W1perm": W1perm,
        "W1permT": np.ascontiguousarray(W1perm.T, np.float32),
        "Acat1": acat1,
        "WcT": np.ascontiguousarray(Wc.reshape(1, C), np.float32),
    }


_RUN_KW = {}
LAST = {}


def kernel(x, edge_index, W0, a_src0, a_dst0, b0, W1, a_src1, a_dst1, b1, Wc, bc):
    x = np.asarray(x)
    edge_index = np.asarray(edge_index)
    per_core, nb, ct, start, stop = _preprocess(edge_index.astype(np.int64))
    nc = build(nb, ct, start, stop)
    shared = _host_inputs(x, np.asarray(W0), np.asarray(W1),
                          np.asarray(a_src0), np.asarray(a_dst0),
                          np.asarray(a_src1), np.asarray(a_dst1), np.asarray(Wc))
    in_maps = [{**shared, **per_core[c]} for c in range(NCORES)]
    res = run_bass_kernel_spmd(nc, in_maps, list(range(NCORES)), **_RUN_KW)
    LAST["res"] = res
    return np.asarray(res.results[0]["out"]).reshape(-1).astype(np.float32)


# revision 4
# speedup vs baseline: 1.0948x; 1.0879x over previous
"""Two-layer GAT on 8 TRN2 cores — redesigned for gather/vector throughput.

Key changes vs baseline kernel.py:
- One merged row gather per edge per layer ([h|as] in-row); ad via one-hot
  matmul from an SBUF table of the core's own dst nodes (no ad gathers).
- One-hot matrices G01/G01T precomputed on host (index-only data), streamed
  densely via HWDGE — no per-chunk is_equal on DVE.
- Host passes xT/W0perm/W1perm layouts so phase A needs no PE transposes;
  phase-A PSUM results are written straight to DRAM with cast DMAs in
  4-tile batches.
- Leaky-relu/exp/1/z moved to the ACT engine (Lrelu/Exp/Ln); msg multiply is
  the only big DVE op per chunk.
- Layer-0 feature order is interleaved (f' = c*8 + h) so the per-edge alpha
  broadcast has unit inner stride.
- Gathers use prepare_only+trigger_dma on rotating SWDGE queues (cfg).
"""

import numpy as np
import ml_dtypes

import concourse.bass as bass
import concourse.tile as tile
from concourse import mybir
from concourse.vector_clock import ScopedClock
from concourse.masks import make_identity
from concourse.bass_utils import run_bass_kernel_spmd
from concourse.library_overlay import lower_extended_insts
from concourse import library_config

F32 = mybir.dt.float32
BF16 = mybir.dt.bfloat16
I16 = mybir.dt.int16
AF = mybir.ActivationFunctionType
ALU = mybir.AluOpType

NCORES = 8
N, E, F_IN, C, H = 20000, 320000, 128, 64, 8
NEG_SLOPE = 0.2
PN = N // NCORES
CHUNK = 128
CPB = 16
EB = CHUNK * CPB
ROW0 = 640            # bf16: [h 512 | as 8 | ad 8 | pad] -> 1280 B rows
ROW1 = 128            # bf16: [h1 64 | as1 | ad1 | pad] -> 256 B rows
NT0 = (N + 127) // 128          # global node tiles (157)
NTL = (PN + 127) // 128         # local dst tiles per core (20)
HROWS = NT0 * 128 + 8           # hcat row allocation (full-tile writes)

CFG = {"mode": "normal", "nq": 4, "pf": 3, "d_ap": False}


class FixedTileContext(tile.TileContext):
    """Walrus here rejects >1 sem wait on the tail Drain: hoist onto NOPs."""

    def _drain_and_barrier(self, tick_clock, wait_clock):
        nop = self.nc.sync.nop(nofuse=True, hint="pre_drain_waits")
        wait_clock.add_sem_waits(nop.ins, ScopedClock({None: tick_clock.global_clock}))
        si = nop.ins.sync_info
        waits = list(si.on_wait) if si and si.on_wait else []
        if len(waits) > 1:
            si.on_wait = [waits[0]]
            for w in waits[1:]:
                n2 = self.nc.sync.nop(nofuse=True, hint="pre_drain_waits")
                n2.ins.sync_info = mybir.SyncInfo(on_wait=[w], on_update=[])
        self.nc.sync.drain()
        self.nc.all_engine_barrier()
        popped = self.nc._tile_sem_poison_stack.pop()
        assert popped is self._sem_poison
        self.nc.clear_and_free_semaphores(list(self.sems.allocated().values()))
        self.nc.all_engine_barrier()


def _legalize_multi_waits(nc, limit=1):
    n_split = 0
    pre = {}
    made = set()
    blocks = [bb for f in nc.m.functions for bb in f.blocks]
    for bb in blocks:
        for inst in list(bb.instructions):
            if inst.name in made:
                continue
            si = inst.sync_info
            waits = list(si.on_wait) if si and si.on_wait else []
            if len(waits) <= limit:
                continue
            si.on_wait = waits[:limit]
            nops = []
            for w in waits[limit:]:
                ni = nc.engines[inst.engine].nop(nofuse=True, hint="wait_split")
                ni.ins.sync_info = mybir.SyncInfo(on_wait=[w], on_update=[])
                nops.append(ni.ins)
                made.add(ni.ins.name)
            pre[(bb.name, inst.name)] = nops
            n_split += len(nops)
    for bb in blocks:
        out = []
        for inst in list(bb.instructions):
            if inst.name in made:
                continue
            out.extend(pre.get((bb.name, inst.name), []))
            out.append(inst)
        bb.instructions = out
    return n_split


def _ap3(base, d1, d2, s1, s2):
    return bass.AP(tensor=base.tensor, offset=base.offset,
                   ap=[base.ap[0], [s1, d1], [s2, d2]])


def _wrap_idx(vals, nb):
    out = np.zeros((128, nb * 128), np.int16)
    for b in range(nb):
        seg = vals[b * EB:(b + 1) * EB].reshape(128, 16).T
        for g in range(8):
            out[g * 16:(g + 1) * 16, b * 128:(b + 1) * 128] = seg
    return out


def _preprocess(edge_index):
    """Partition edges by dst core/tile with a chunk schedule common to all
    cores; emit src-idx tables and dense one-hot [G01 | G01T] chunks."""
    src = np.concatenate([edge_index[0], np.arange(N, dtype=np.int64)])
    dst = np.concatenate([edge_index[1], np.arange(N, dtype=np.int64)])
    raw = []
    for c in range(NCORES):
        m = (dst >= c * PN) & (dst < (c + 1) * PN)
        s, dl = src[m], dst[m] - c * PN
        tid = dl // 128
        raw.append([(s[tid == t], dl[tid == t] - t * 128) for t in range(NTL)])
    kt = [max((len(raw[c][t][0]) + CHUNK - 1) // CHUNK for c in range(NCORES))
          for t in range(NTL)]
    nch = sum(kt)
    nch_p = (nch + CPB - 1) // CPB * CPB
    nb = nch_p // CPB
    ct = sum(([t] * kt[t] for t in range(NTL)), []) + [NTL - 1] * (nch_p - nch)
    start = [i == 0 or ct[i] != ct[i - 1] for i in range(nch_p)]
    stop = [i == nch_p - 1 or ct[i + 1] != ct[i] for i in range(nch_p)]
    per_core = []
    for c in range(NCORES):
        s_l, d_l = [], []
        for t in range(NTL):
            st, dt_ = raw[c][t]
            pad = kt[t] * CHUNK - len(st)
            s_l.append(np.concatenate([st, np.full(pad, N, np.int64)]))
            d_l.append(np.concatenate([dt_, -np.ones(pad, np.int64)]))
        pad = (nch_p - nch) * CHUNK
        s_ = np.concatenate(s_l + [np.full(pad, N, np.int64)])
        d_ = np.concatenate(d_l + [-np.ones(pad, np.int64)])
        g01 = np.zeros((nch_p, 128, 256), ml_dtypes.bfloat16)
        ei = np.arange(nch_p * 128)
        real = d_ >= 0
        ch, ep, dp = ei[real] // 128, ei[real] % 128, d_[real].astype(np.int64)
        g01[ch, ep, dp] = 1
        g01[ch, dp, 128 + ep] = 1
        per_core.append({
            "idx_main": _wrap_idx(s_.astype(np.int16), nb),
            "g01cat": g01,
        })
    return per_core, nb, ct, start, stop


def build(nb, ct, start, stop):
    nch = nb * CPB
    mode, NQ, PF = CFG["mode"], CFG["nq"], CFG["pf"]
    nc = bass.Bass(num_devices=NCORES, num_swdge_queues=NQ)

    xt_in = nc.declare_dram_parameter("xT", [F_IN, N], F32, isOutput=False)
    w0p_in = nc.declare_dram_parameter("W0perm", [F_IN, 512], F32, isOutput=False)
    w0pt_in = nc.declare_dram_parameter("W0permT", [512, F_IN], F32, isOutput=False)
    ac0_in = nc.declare_dram_parameter("Acat0p", [512, 16], F32, isOutput=False)
    w1p_in = nc.declare_dram_parameter("W1perm", [512, C], F32, isOutput=False)
    w1pt_in = nc.declare_dram_parameter("W1permT", [C, 512], F32, isOutput=False)
    ac1_in = nc.declare_dram_parameter("Acat1", [C, 2], F32, isOutput=False)
    wct_in = nc.declare_dram_parameter("WcT", [1, C], F32, isOutput=False)
    im_in = nc.declare_dram_parameter("idx_main", [128, nb * 128], I16, isOutput=False)
    g01_in = nc.declare_dram_parameter("g01cat", [nch, 128, 256], BF16, isOutput=False)
    out_fin = nc.declare_dram_parameter("out", [1, 1], F32, isOutput=True)
    dbg_hrow = nc.declare_dram_parameter("dbg_hrow", [128, ROW0], BF16, isOutput=True)
    dbg_adt = nc.declare_dram_parameter("dbg_adt", [128, NTL * 8], BF16, isOutput=True)
    dbg_h1own = nc.declare_dram_parameter("dbg_h1own", [128, ROW1], BF16, isOutput=True)
    dbg_ck = nc.declare_dram_parameter("dbg_ck", [128, 48], F32, isOutput=True)

    hcat0 = nc.dram_tensor("hcat0", [HROWS, ROW0], BF16)
    h1own = nc.dram_tensor("h1own", [NTL * 128, ROW1], BF16)
    h1tab = nc.dram_tensor("h1tab", [N + 8, ROW1], BF16, addr_space="Shared")
    pool_src = nc.dram_tensor("pool_src", [1, C], F32)
    pool_red = nc.dram_tensor("pool_red", [1, C], F32, addr_space="Shared")

    nc.gpsimd.load_library(library_config.mlp)
    eb_reg = nc.gpsimd.to_reg(EB)

    hc_all = bass.AP(tensor=hcat0[:, :].tensor, offset=0,
                     ap=[[ROW0, N + 1], [1, ROW0]])
    h1_all = bass.AP(tensor=h1tab[:, :].tensor, offset=0,
                     ap=[[ROW1, N + 1], [1, ROW1]])

    with FixedTileContext(nc) as tc:
        with tc.tile_pool(name="sg", bufs=1) as sg, \
             tc.tile_pool(name="op", bufs=3) as op, \
             tc.tile_pool(name="wk", bufs=3) as wk, \
             tc.tile_pool(name="ep", bufs=2) as ep:

            psW_cm = tc.tile_pool(name="psW", bufs=2, space="PSUM")
            psW = psW_cm.__enter__()
            psH_cm = tc.tile_pool(name="psH", bufs=3, space="PSUM")
            psH = psH_cm.__enter__()
            psE_cm = tc.tile_pool(name="psE", bufs=3, space="PSUM")
            psE = psE_cm.__enter__()
            # ---------- weights ----------
            ident = sg.tile([128, 128], BF16)
            make_identity(nc, ident[:])
            eps_sb = sg.tile([128, 1], F32)
            nc.vector.memset(eps_sb[:], 1e-20)
            m1_sb = sg.tile([128, 1], F32)
            nc.vector.memset(m1_sb[:], -1.0)
            ones_sb = sg.tile([128, 1], BF16)
            nc.vector.memset(ones_sb[:], 1.0)
            wct_sb = sg.tile([1, C], F32)
            nc.sync.dma_start(out=wct_sb[:], in_=wct_in[:])
            idxm_sb = sg.tile([128, nb * 128], I16)
            nc.sync.dma_start(out=idxm_sb[:], in_=im_in[:])

            w0cat = sg.tile([128, 528], BF16)
            nc.gpsimd.dma_start(out=w0cat[:, 0:512], in_=w0p_in[:])
            w0pt = sg.tile([128, 4, 128], BF16)
            nc.gpsimd.dma_start(
                out=w0pt[:],
                in_=bass.AP(tensor=w0pt_in[:, :].tensor, offset=0,
                            ap=[[128, 128], [128 * 128, 4], [1, 128]]))
            ac0 = sg.tile([128, 4, 16], BF16)
            nc.gpsimd.dma_start(
                out=ac0[:],
                in_=bass.AP(tensor=ac0_in[:, :].tensor, offset=0,
                            ap=[[16, 128], [16 * 128, 4], [1, 16]]))
            wext = psW.tile([128, 16], F32, space="PSUM", tag="wx")
            for q in range(4):
                nc.tensor.matmul(out=wext[:], lhsT=w0pt[:, q, :], rhs=ac0[:, q, :],
                                 start=(q == 0), stop=(q == 3))
            nc.vector.tensor_copy(out=w0cat[:, 512:528], in_=wext[:])

            w1cat = sg.tile([128, 4, C + 2], BF16)
            nc.gpsimd.dma_start(
                out=bass.AP(tensor=w1cat[:].tensor, offset=w1cat[:].offset,
                            ap=[w1cat[:].ap[0], [C + 2, 4], [1, C]]),
                in_=bass.AP(tensor=w1p_in[:, :].tensor, offset=0,
                            ap=[[C, 128], [C * 128, 4], [1, C]]))
            w1pt = sg.tile([C, 4, 128], BF16)
            nc.gpsimd.dma_start(
                out=w1pt[:],
                in_=bass.AP(tensor=w1pt_in[:, :].tensor, offset=0,
                            ap=[[512, C], [128, 4], [1, 128]]))
            ac1 = sg.tile([C, 2], BF16)
            nc.gpsimd.dma_start(out=ac1[:], in_=ac1_in[:])
            for q in range(4):
                w1e = psW.tile([128, 2], F32, space="PSUM", tag="wx")
                nc.tensor.matmul(out=w1e[:], lhsT=w1pt[:, q, :], rhs=ac1[:],
                                 start=True, stop=True)
                nc.vector.tensor_copy(out=w1cat[:, q, C:C + 2], in_=w1e[:])

            # ---------- phase A: hcat0 table (replicated) ----------
            xp_cm = tc.tile_pool(name="xp", bufs=1)
            xp = xp_cm.__enter__()
            xt_sb = xp.tile([128, NT0 * 128], BF16)
            for xq in range(4):
                nc.gpsimd.dma_start(out=xt_sb[:, xq * 5000:(xq + 1) * 5000],
                                    in_=xt_in[:, xq * 5000:(xq + 1) * 5000])
            nc.vector.memset(xt_sb[:, N:], 0.0)
            zrow = sg.tile([1, ROW0], BF16)
            nc.vector.memset(zrow[:], 0.0)
            nc.sync.dma_start(out=hcat0[N:N + 1, :], in_=zrow[:])

            for g4 in range((NT0 + 3) // 4):
                ts = list(range(g4 * 4, min(g4 * 4 + 4, NT0)))
                k4 = len(ts)
                stg = wk.tile([128, 4, 528], BF16, tag="stg")
                for k, t in enumerate(ts):
                    hp = psH.tile([128, 512], F32, space="PSUM", tag="hp")
                    epi = psE.tile([128, 16], F32, space="PSUM", tag="ep16")
                    nc.tensor.matmul(out=hp[:], lhsT=xt_sb[:, t * 128:(t + 1) * 128],
                                     rhs=w0cat[:, 0:512], start=True, stop=True)
                    nc.tensor.matmul(out=epi[:], lhsT=xt_sb[:, t * 128:(t + 1) * 128],
                                     rhs=w0cat[:, 512:528], start=True, stop=True)
                    if k % 2 == 0:
                        nc.vector.tensor_copy(out=stg[:, k, 0:512], in_=hp[:])
                        nc.scalar.activation(out=stg[:, k, 512:528], in_=epi[:],
                                             func=AF.Copy)
                    else:
                        nc.scalar.activation(out=stg[:, k, 0:512], in_=hp[:],
                                             func=AF.Copy)
                        nc.vector.tensor_copy(out=stg[:, k, 512:528], in_=epi[:])
                r0 = g4 * 4 * 128
                nc.sync.dma_start(
                    out=bass.AP(tensor=hcat0[:, :].tensor, offset=r0 * ROW0,
                                ap=[[ROW0, 128], [128 * ROW0, k4], [1, 528]]),
                    in_=stg[:, 0:k4, :])

            xp_cm.__exit__(None, None, None)

            # ---------- per-core ad table readback ----------
            pid = nc.sync.partition_id()
            adt = sg.tile([128, NTL, 8], BF16)
            nc.sync.dma_start(
                out=adt[:],
                in_=bass.AP(tensor=hcat0[:, :].tensor,
                            offset=pid * (PN * ROW0) + 520,
                            ap=[[ROW0, 128], [128 * ROW0, NTL], [1, 8]]))

            psE_cm.__exit__(None, None, None)
            psH_cm.__exit__(None, None, None)
            psW_cm.__exit__(None, None, None)
            psA_cm = tc.tile_pool(name="psA", bufs=2, space="PSUM")
            psA = psA_cm.__enter__()
            psB_cm = tc.tile_pool(name="psB", bufs=1, space="PSUM")
            psB = psB_cm.__enter__()
            psT_cm = tc.tile_pool(name="psT", bufs=1, space="PSUM")
            psT = psT_cm.__enter__()
            psC_cm = tc.tile_pool(name="psC", bufs=2, space="PSUM")
            psC = psC_cm.__enter__()

            nc.sync.dma_start(out=dbg_hrow[:, :], in_=hcat0[0:128, :])
            nc.sync.dma_start(
                out=dbg_adt[:, :],
                in_=bass.AP(tensor=adt[:].tensor, offset=adt[:].offset,
                            ap=[adt[:].ap[0], [1, NTL * 8]]))

            # ---------- phase B: layer-0 aggregation + h1 rows ----------
            gp_cm = tc.tile_pool(name="gp", bufs=4)
            gp = gp_cm.__enter__()
            dsem = [nc.alloc_semaphore(f"dsB{q}") for q in range(NQ)]

            def issue_gather(b, pool_tag, elem, table_ap, sems):
                g = gp.tile([128, CPB, elem], BF16, tag=pool_tag)
                q = b % NQ
                if mode == "prep":
                    nc.gpsimd.dma_gather(
                        out_ap=g[:], in_ap=table_ap,
                        idxs_ap=idxm_sb[:, b * 128:(b + 1) * 128],
                        num_idxs=EB, num_idxs_reg=eb_reg, elem_size=elem,
                        single_packet=False, prepare_only=True, sem=sems[q],
                        queue_num=q)
                    nc.gpsimd.trigger_dma(count=None, queue_num=q)
                else:
                    nc.gpsimd.dma_gather(
                        out_ap=g[:], in_ap=table_ap,
                        idxs_ap=idxm_sb[:, b * 128:(b + 1) * 128],
                        num_idxs=EB, num_idxs_reg=eb_reg, elem_size=elem,
                        single_packet=False, queue_num=q)
                return g

            def load_g01(b):
                t = op.tile([128, CPB, 256], BF16, tag="g01")
                nc.sync.dma_start(
                    out=t[:],
                    in_=bass.AP(tensor=g01_in[:, :, :].tensor, offset=b * CPB * 128 * 256,
                                ap=[[256, 128], [128 * 256, CPB], [1, 256]]))
                return t

            g0_tiles = {b: issue_gather(b, "g0", ROW0, hc_all, dsem) for b in range(min(PF, nb))}
            g01_tiles = {b: load_g01(b) for b in range(min(PF, nb))}
            aggp = zp = None
            for b in range(nb):
                if b + PF < nb:
                    g0_tiles[b + PF] = issue_gather(b + PF, "g0", ROW0, hc_all, dsem)
                    g01_tiles[b + PF] = load_g01(b + PF)
                g0 = g0_tiles.pop(b)
                go = g01_tiles.pop(b)
                for cpos in range(CPB):
                    i = b * CPB + cpos
                    t = ct[i]
                    if start[i]:
                        aggp = psA.tile([128, 512], F32, space="PSUM", tag="agg")
                        zp = psB.tile([128, 8], F32, space="PSUM", tag="z")
                    adp = psC.tile([128, 8], F32, space="PSUM", tag="adp")
                    nc.tensor.matmul(out=adp[:], lhsT=go[:, cpos, 128:256],
                                     rhs=adt[:, t, :], start=True, stop=True)
                    e0 = wk.tile([128, 8], F32, tag="e0")
                    nc.vector.tensor_tensor(out=e0[:], in0=adp[:],
                                            in1=g0[:, cpos, 512:520], op=ALU.add)
                    lr = wk.tile([128, 8], F32, tag="lr")
                    nc.scalar.activation(out=lr[:], in_=e0[:], func=AF.Prelu,
                                         alpha=NEG_SLOPE)
                    pbf = wk.tile([128, 8], BF16, tag="pbf")
                    nc.scalar.activation(out=pbf[:], in_=lr[:], func=AF.Exp)
                    if i == 0:
                        ckb = wk.tile([128, 48], F32, tag="ckb")
                        nc.vector.tensor_copy(out=ckb[:, 0:8], in_=adp[:])
                        nc.vector.tensor_copy(out=ckb[:, 8:16], in_=e0[:])
                        nc.vector.tensor_copy(out=ckb[:, 16:24], in_=lr[:])
                        nc.vector.tensor_copy(out=ckb[:, 24:32], in_=pbf[:])
                        nc.vector.tensor_copy(out=ckb[:, 32:40], in_=g0[:, cpos, 512:520])
                        nc.vector.tensor_copy(out=ckb[:, 40:48], in_=g0[:, cpos, 0:8])
                        nc.sync.dma_start(out=dbg_ck[:, :], in_=ckb[:])
                    msg = wk.tile([128, 512], BF16, tag="msg")
                    nc.vector.tensor_tensor(out=msg[:], in0=g0[:, cpos, 0:512],
                                            in1=_ap3(pbf[:], 64, 8, 0, 1), op=ALU.mult)
                    nc.tensor.matmul(out=aggp[:], lhsT=go[:, cpos, 0:128], rhs=msg[:],
                                     start=start[i], stop=stop[i])
                    nc.tensor.matmul(out=zp[:], lhsT=go[:, cpos, 0:128], rhs=pbf[:],
                                     start=start[i], stop=stop[i])
                    if stop[i]:
                        rows = min(128, PN - t * 128)
                        zli = ep.tile([128, 8], F32, tag="zli")
                        nc.scalar.activation(out=zli[:], in_=zp[:], func=AF.Ln, bias=eps_sb[:])
                        zinv = ep.tile([128, 8], BF16, tag="zinv")
                        nc.scalar.activation(out=zinv[:], in_=zli[:], func=AF.Exp,
                                             scale=-1.0)
                        u = ep.tile([128, 512], BF16, tag="u")
                        nc.vector.tensor_tensor(out=u[:], in0=aggp[:],
                                                in1=_ap3(zinv[:], 64, 8, 0, 1),
                                                op=ALU.mult)
                        t1 = ep.tile([128, 512], BF16, tag="t1")
                        nc.scalar.activation(out=t1[:], in_=u[:], func=AF.Relu,
                                             scale=-1.0)
                        t2 = ep.tile([128, 512], BF16, tag="t2")
                        nc.scalar.activation(out=t2[:], in_=t1[:], func=AF.Exp,
                                             scale=-1.0)
                        t3 = ep.tile([128, 512], BF16, tag="t3")
                        nc.scalar.activation(out=t3[:], in_=t2[:], func=AF.Prelu,
                                             bias=m1_sb[:], alpha=1.0)
                        h1in = ep.tile([128, 512], BF16, tag="h1in")
                        nc.vector.tensor_tensor(out=h1in[:], in0=u[:], in1=t3[:],
                                                op=ALU.max)
                        h1t = ep.tile([128, 4, 128], BF16, tag="h1t")
                        for q in range(4):
                            tp = psT.tile([128, 128], BF16, space="PSUM", tag="tp")
                            nc.tensor.transpose(out=tp[:, :rows],
                                                in_=h1in[:rows, q * 128:(q + 1) * 128],
                                                identity=ident[:rows, :rows])
                            nc.scalar.activation(out=h1t[:, q, :rows], in_=tp[:, :rows],
                                                 func=AF.Copy)
                        h1ps = psB.tile([128, C + 2], F32, space="PSUM", tag="h1")
                        for q in range(4):
                            nc.tensor.matmul(out=h1ps[:rows, :], lhsT=h1t[:, q, :rows],
                                             rhs=w1cat[:, q, :], start=(q == 0),
                                             stop=(q == 3))
                        h1row = ep.tile([128, ROW1], BF16, tag="h1row")
                        nc.vector.memset(h1row[:, C + 2:ROW1], 0.0)
                        nc.vector.tensor_copy(out=h1row[:rows, 0:C + 2],
                                              in_=h1ps[:rows, :])
                        nc.sync.dma_start(out=h1own[t * 128:t * 128 + rows, :],
                                          in_=h1row[:rows, :])

            gp_cm.__exit__(None, None, None)
            nc.sync.dma_start(out=dbg_h1own[:, :], in_=h1own[0:128, :])

            # ---------- phase C: AllGather + ad1 readback ----------
            zrow1 = sg.tile([8, ROW1], BF16)
            nc.vector.memset(zrow1[:], 0.0)
            nc.sync.dma_start(out=h1tab[N:N + 8, :], in_=zrow1[:])
            nc.gpsimd.collective_compute(
                "AllGather", ALU.bypass, replica_groups=[list(range(NCORES))],
                ins=[h1own[0:PN, :]], outs=[h1tab[0:N, :]])
            ad1t = sg.tile([128, NTL, 1], BF16)
            nc.sync.dma_start(
                out=ad1t[:],
                in_=bass.AP(tensor=h1own[:, :].tensor, offset=C + 1,
                            ap=[[ROW1, 128], [128 * ROW1, NTL], [1, 1]]))

            # ---------- phase D: layer-1 aggregation + pooling ----------
            d_ap = CFG["d_ap"]
            if d_ap:
                nc.gpsimd.load_library(library_config.ap_gather)
            pool_acc = sg.tile([1, C], F32)
            nc.vector.memset(pool_acc[:], 0.0)
            with tc.tile_pool(name="dp", bufs=1) as dp, \
                 tc.tile_pool(name="dg", bufs=3) as dg:
                if d_ap:
                    tab1 = dp.tile([128, 20000, 2], BF16)
                    nc.sync.dma_start(
                        out=bass.AP(tensor=tab1[:].tensor, offset=tab1[:].offset,
                                    ap=[tab1[:].ap[0], [2, 20000], [1, 1]]),
                        in_=h1tab[0:N, :], transpose=True)

                def issue_apg(b):
                    if not d_ap:
                        g = dg.tile([128, CPB, ROW1], BF16, tag="g1t")
                        nc.gpsimd.dma_gather(
                            out_ap=g[:], in_ap=h1_all,
                            idxs_ap=idxm_sb[:, b * 128:(b + 1) * 128],
                            num_idxs=EB, num_idxs_reg=eb_reg, elem_size=ROW1,
                            single_packet=False, queue_num=b % NQ)
                        return g
                    g = dg.tile([128, EB, 2], BF16, tag="g1t")
                    nc.gpsimd.ap_gather(
                        out_ap=g[:], in_ap=tab1[:],
                        idxs_ap=idxm_sb[:, b * 128:(b + 1) * 128],
                        channels=128, num_elems=20000, d=2, num_idxs=EB)
                    return g

                g1_tiles = {b: issue_apg(b) for b in range(min(PF, nb))}
                g01_tiles = {b: load_g01(b) for b in range(min(PF, nb))}
                for b in range(nb):
                    if b + PF < nb:
                        g1_tiles[b + PF] = issue_apg(b + PF)
                        g01_tiles[b + PF] = load_g01(b + PF)
                    g1 = g1_tiles.pop(b)
                    go = g01_tiles.pop(b)
                    for cpos in range(CPB):
                        i = b * CPB + cpos
                        t = ct[i]
                        if start[i]:
                            aggp = psA.tile([128, C], F32, space="PSUM", tag="agg")
                            zp = psB.tile([128, 1], F32, space="PSUM", tag="z")
                        if d_ap:
                            tp1 = psT.tile([128, 128], BF16, space="PSUM", tag="tp")
                            nc.tensor.transpose(
                                out=tp1[:],
                                in_=g1[:, cpos * 128:(cpos + 1) * 128, 0:1],
                                identity=ident[:])
                            h1src = tp1
                            as1v = tp1[:, C:C + 1]
                        else:
                            h1src = None
                            as1v = g1[:, cpos, C:C + 1]
                        as1s = wk.tile([128, 1], BF16, tag="as1")
                        nc.vector.tensor_copy(out=as1s[:], in_=as1v)
                        adp = psC.tile([128, 1], F32, space="PSUM", tag="adp")
                        nc.tensor.matmul(out=adp[:], lhsT=go[:, cpos, 128:256],
                                         rhs=ad1t[:, t, :], start=True, stop=True)
                        e0 = wk.tile([128, 1], F32, tag="e1")
                        nc.vector.tensor_tensor(out=e0[:], in0=adp[:],
                                                in1=as1s[:], op=ALU.add)
                        lr = wk.tile([128, 1], F32, tag="lr1")
                        nc.scalar.activation(out=lr[:], in_=e0[:], func=AF.Prelu,
                                             alpha=NEG_SLOPE)
                        pbf = wk.tile([128, 1], BF16, tag="pbf1")
                        nc.scalar.activation(out=pbf[:], in_=lr[:], func=AF.Exp)
                        msg = wk.tile([128, C], BF16, tag="msg1")
                        m_in0 = h1src[:, 0:C] if d_ap else g1[:, cpos, 0:C]
                        nc.vector.tensor_tensor(out=msg[:], in0=m_in0,
                                                in1=_ap3(pbf[:], 1, C, 1, 0),
                                                op=ALU.mult)
                        nc.tensor.matmul(out=aggp[:], lhsT=go[:, cpos, 0:128], rhs=msg[:],
                                         start=start[i], stop=stop[i])
                        nc.tensor.matmul(out=zp[:], lhsT=go[:, cpos, 0:128], rhs=pbf[:],
                                         start=start[i], stop=stop[i])
                        if stop[i]:
                            rows = min(128, PN - t * 128)
                            zli = ep.tile([128, 1], F32, tag="zl1")
                            nc.scalar.activation(out=zli[:], in_=zp[:], func=AF.Ln,
                                                 bias=eps_sb[:])
                            zinv = ep.tile([128, 1], BF16, tag="zi1")
                            nc.scalar.activation(out=zinv[:], in_=zli[:], func=AF.Exp,
                                                 scale=-1.0)
                            o1 = ep.tile([128, C], BF16, tag="o1")
                            nc.vector.tensor_tensor(out=o1[:], in0=aggp[:],
                                                    in1=_ap3(zinv[:], 1, C, 1, 0),
                                                    op=ALU.mult)
                            pps = psB.tile([1, C], F32, space="PSUM", tag="pool")
                            nc.tensor.matmul(out=pps[:], lhsT=ones_sb[:rows, :],
                                             rhs=o1[:rows, :], start=True, stop=True)
                            nc.vector.tensor_tensor(out=pool_acc[:], in0=pool_acc[:],
                                                    in1=pps[:], op=ALU.add)

            # ---------- final: AllReduce, logit, sigmoid ----------
            nc.sync.dma_start(out=pool_src[:, :], in_=pool_acc[:])
            nc.gpsimd.collective_compute(
                "AllReduce", ALU.add, replica_groups=[list(range(NCORES))],
                ins=[pool_src[:, :]], outs=[pool_red[:, :]])
            pr = sg.tile([1, C], F32)
            nc.sync.dma_start(out=pr[:], in_=pool_red[:, :])
            tmul = sg.tile([1, C], F32)
            nc.vector.tensor_tensor(out=tmul[:], in0=pr[:], in1=wct_sb[:], op=ALU.mult)
            sres = sg.tile([1, 1], F32)
            nc.vector.tensor_reduce(out=sres[:], in_=tmul[:], axis=mybir.AxisListType.X,
                                    op=ALU.add)
            nc.vector.tensor_scalar(out=sres[:], in0=sres[:], scalar1=-1.0 / N,
                                    scalar2=None, op0=ALU.mult)
            nc.scalar.activation(out=sres[:], in_=sres[:], func=AF.Exp)
            nc.vector.tensor_scalar(out=sres[:], in0=sres[:], scalar1=1.0,
                                    scalar2=None, op0=ALU.add)
            nc.vector.reciprocal(out=sres[:], in_=sres[:])
            nc.sync.dma_start(out=out_fin[:, :], in_=sres[:])
            psC_cm.__exit__(None, None, None)
            psT_cm.__exit__(None, None, None)
            psB_cm.__exit__(None, None, None)
            psA_cm.__exit__(None, None, None)

    ns = _legalize_multi_waits(nc)
    print(f"[kernel_new] split {ns} excess sem waits onto nops")
    nc.finalize()
    lower_extended_insts(nc)
    return nc


def _host_inputs(x, W0, W1, a_src0, a_dst0, a_src1, a_dst1, Wc):
    """Layout-only input transforms. Layer-0 features are permuted to
    interleaved order f' = c*8 + h (h innermost)."""
    perm0 = np.empty(512, np.int64)            # perm0[f'] = original col
    for h in range(H):
        for c in range(C):
            perm0[c * H + h] = h * C + c
    W0perm = np.ascontiguousarray(x.dtype.type(0) + W0[:, perm0], np.float32)
    acat0p = np.zeros((512, 16), np.float32)
    for h in range(H):
        for c in range(C):
            acat0p[c * H + h, h] = a_src0[h, c]
            acat0p[c * H + h, 8 + h] = a_dst0[h, c]
    W1perm = np.ascontiguousarray(W1[perm0, :], np.float32)
    acat1 = np.zeros((C, 2), np.float32)
    acat1[:, 0] = a_src1[0]
    acat1[:, 1] = a_dst1[0]
    return {
        "xT": np.ascontiguousarray(x.T, np.float32),
        "W0perm": W0perm,
        "W0permT": np.ascontiguousarray(W0perm.T, np.float32),
        "Acat0p": acat0p,
        "W1perm": W1perm,
        "W1permT": np.ascontiguousarray(W1perm.T, np.float32),
        "Acat1": acat1,
        "WcT": np.ascontiguousarray(Wc.reshape(1, C), np.float32),
    }


_RUN_KW = {}
LAST = {}


def kernel(x, edge_index, W0, a_src0, a_dst0, b0, W1, a_src1, a_dst1, b1, Wc, bc):
    x = np.asarray(x)
    edge_index = np.asarray(edge_index)
    per_core, nb, ct, start, stop = _preprocess(edge_index.astype(np.int64))
    nc = build(nb, ct, start, stop)
    shared = _host_inputs(x, np.asarray(W0), np.asarray(W1),
                          np.asarray(a_src0), np.asarray(a_dst0),
                          np.asarray(a_src1), np.asarray(a_dst1), np.asarray(Wc))
    in_maps = [{**shared, **per_core[c]} for c in range(NCORES)]
    res = run_bass_kernel_spmd(nc, in_maps, list(range(NCORES)), **_RUN_KW)
    LAST["res"] = res
    return np.asarray(res.results[0]["out"]).reshape(-1).astype(np.float32)
